# revision 2
# baseline (speedup 1.0000x reference)
"""Trainium2 Bass kernel for nn_CatEmbedder (gnn_message_passing).

Takes FULL inputs, shards batch B=32768 across 8 NeuronCores (4096 each),
replicates the embedding table + weights, runs an SPMD Bass kernel, and
concatenates the per-core outputs.

Per-core pipeline (32 blocks x 128 samples):
  1. indirect-DMA gather: emb[p, f*64:(f+1)*64] = table[idx[p,f]]  ([128,3200])
  2. PE transposes of [128,128] chunks (2 fields each) -> PSUM -> SBUF (et)
  3. squares (ACT/DVE split) into interleaved sq chunks
  4. PE seg-matmuls: field-sum + field-sum-of-squares  -> [64,256] PSUM
  5. PE u-matmuls: u_f = (S + PROBE*support_f)/c + ga_b  (bias via aug row)
  6. ACT relu-evict (bf16) -> PE accumulates sum_f relu(u_f)
  7. transposed MLPs for global/local branches, combine, transpose back, store
"""

import os
import sys
import numpy as np

sys.path.insert(0, "/opt/trn_rl_repo")

# ---- problem constants (hardcoded per the contract) ----
B, F, D, NCT = 32768, 50, 64, 100000
PROBE, ALPHA = 39.0, 0.5
NF = F + 1              # 51 fields
CD = NF + PROBE         # 90.0
NCORES = 8
BS = B // NCORES        # 4096 samples per core
BLK = 128
SUPER = 8               # idx/numf superblock (blocks per DMA)
NBLK_FULL = BS // BLK   # 32

USE_F32R = False        # fast fp32 matmul mode (walrus needs fp32r-typed producers)

_CACHE = {}


def _build(nblk=NBLK_FULL, reps=1):
    import concourse.bass as bass
    import concourse.mybir as mybir
    import concourse.tile as tile
    from concourse import bacc
    from contextlib import ExitStack

    f32 = mybir.dt.float32
    f32r = mybir.dt.float32r
    bf16 = mybir.dt.bfloat16
    i32 = mybir.dt.int32
    AL = mybir.AluOpType
    AF = mybir.ActivationFunctionType

    def r(ap):  # fp32 -> fp32r view for fast matmuls
        return ap.bitcast(f32r) if USE_F32R else ap

    nc = bacc.Bacc(None)

    # ---- DRAM parameters (order matters only for debug; bound by name) ----
    idx_d = nc.declare_dram_parameter("cat_idx", [BS, F], i32, isOutput=False)
    numf_d = nc.declare_dram_parameter("numf", [BS], f32, isOutput=False)
    table_d = nc.declare_dram_parameter("table", [NCT, D], f32, isOutput=False)
    ident_d = nc.declare_dram_parameter("ident128", [128, 128], f32, isOutput=False)
    segf_d = nc.declare_dram_parameter("seg_f", [128, D], f32, isOutput=False)
    segb_d = nc.declare_dram_parameter("seg_b", [128, D], bf16, isOutput=False)
    i64b_d = nc.declare_dram_parameter("i64_b", [D, D], bf16, isOutput=False)
    i64f_d = nc.declare_dram_parameter("i64_f", [D, D], f32, isOutput=False)
    waug_d = nc.declare_dram_parameter("waug", [D + 1, 128], f32, isOutput=False)
    gw2_d = nc.declare_dram_parameter("gw2", [128, 128], f32, isOutput=False)
    g0_d = nc.declare_dram_parameter("g0T", [D, D], f32, isOutput=False)
    g1_d = nc.declare_dram_parameter("g1aug", [D + 1, D], f32, isOutput=False)
    l0_d = nc.declare_dram_parameter("l0T", [D, D], f32, isOutput=False)
    l1_d = nc.declare_dram_parameter("l1aug", [D + 1, D], f32, isOutput=False)
    cols_d = nc.declare_dram_parameter("cols", [D, 4], f32, isOutput=False)
    ones_d = nc.declare_dram_parameter("ones164", [1, D], f32, isOutput=False)
    out_d = nc.declare_dram_parameter("out", [BS, D], f32, isOutput=True)

    GROUPS = [(0, 4), (4, 4), (8, 4), (12, 4), (16, 4), (20, 4), (24, 1)]
    NCHUNK = 25  # 25 chunks of 128 cols (2 fields each)

    with tile.TileContext(nc) as tc, ExitStack() as ctx:
        const = ctx.enter_context(tc.tile_pool(name="const", bufs=1))
        sb = ctx.enter_context(tc.tile_pool(name="sb", bufs=2))
        pst = ctx.enter_context(tc.tile_pool(name="pst", bufs=2, space="PSUM"))
        psu = ctx.enter_context(tc.tile_pool(name="psu", bufs=2, space="PSUM"))
        pseg = ctx.enter_context(tc.tile_pool(name="pseg", bufs=1, space="PSUM"))
        pracc = ctx.enter_context(tc.tile_pool(name="pracc", bufs=1, space="PSUM"))
        psm = ctx.enter_context(tc.tile_pool(name="psm", bufs=2, space="PSUM"))

        # ---- load constants once ----
        ident_t = const.tile([128, 128], f32)
        nc.sync.dma_start(ident_t[:], ident_d[:])
        segf_t = const.tile([128, D], f32)
        nc.sync.dma_start(segf_t[:], segf_d[:])
        segb_t = const.tile([128, D], bf16)
        nc.sync.dma_start(segb_t[:], segb_d[:])
        i64b_t = const.tile([D, D], bf16)
        nc.sync.dma_start(i64b_t[:], i64b_d[:])
        i64f_t = const.tile([D, D], f32)
        nc.sync.dma_start(i64f_t[:], i64f_d[:])
        waug_t = const.tile([D + 1, 128], f32)
        nc.sync.dma_start(waug_t[:], waug_d[:])
        gw2_t = const.tile([128, 128], f32)
        nc.sync.dma_start(gw2_t[:], gw2_d[:])
        g0_t = const.tile([D, D], f32)
        nc.sync.dma_start(g0_t[:], g0_d[:])
        g1_t = const.tile([D + 1, D], f32)
        nc.sync.dma_start(g1_t[:], g1_d[:])
        l0_t = const.tile([D, D], f32)
        nc.sync.dma_start(l0_t[:], l0_d[:])
        l1_t = const.tile([D + 1, D], f32)
        nc.sync.dma_start(l1_t[:], l1_d[:])
        cols_t = const.tile([D, 4], f32)
        nc.sync.dma_start(cols_t[:], cols_d[:])
        ones_t = const.tile([1, D], f32)
        nc.sync.dma_start(ones_t[:], ones_d[:])
        onesrow_t = const.tile([1, 128], f32)
        nc.vector.memset(onesrow_t[:], 1.0)

        numw_c = cols_t[:, 0:1]
        numb_c = cols_t[:, 1:2]
        gb0_c = cols_t[:, 2:3]
        lb0_c = cols_t[:, 3:4]

        idx_view = idx_d[:].rearrange("(s k p) f -> s p k f", p=BLK, k=SUPER)

        idx_t = None
        numf_t = None
        rep_cm = tc.For_i(0, reps, 1) if reps > 1 else None
        if rep_cm is not None:
            rep_cm.__enter__()
        for blk in range(nblk):
            s = blk % SUPER
            if s == 0:
                si = blk // SUPER
                idx_t = sb.tile([128, SUPER * F], i32, tag="idx")
                nc.sync.dma_start(
                    idx_t[:].rearrange("p (k f) -> p k f", k=SUPER), idx_view[si]
                )
                numf_t = sb.tile([1, SUPER * BLK], f32, tag="numf")
                nc.sync.dma_start(
                    numf_t[:], numf_d[None, si * SUPER * BLK:(si + 1) * SUPER * BLK]
                )

            # ---- 1. gather (one indirect DMA per field: HW consumes one
            # index per output partition) ----
            emb = sb.tile([128, F * D], f32, tag="emb")
            for f in range(F):
                nc.gpsimd.indirect_dma_start(
                    out=emb[:, f * D:(f + 1) * D],
                    out_offset=None,
                    in_=table_d[:, :],
                    in_offset=bass.IndirectOffsetOnAxis(
                        ap=idx_t[:, s * F + f:s * F + f + 1], axis=0
                    ),
                )

            # ---- numeric-field embedding (transposed): num_embT [64, 128] ----
            nrep = psm.tile([D, 128], f32, tag="small")
            nc.tensor.matmul(
                out=nrep[:], lhsT=ones_t[:],
                rhs=numf_t[:, s * BLK:(s + 1) * BLK],
                start=True, stop=True,
            )
            numembT = sb.tile([D, 128], f32, tag="numembT")
            nc.scalar.activation(
                out=numembT[:], in_=nrep[:], func=AF.Identity,
                bias=numb_c, scale=numw_c,
            )

            # ---- 2. transposes + evict; 3. squares ----
            # et layout: [128, 25*256] chunks [embT(128) | sq(128)]
            et = sb.tile([128, NCHUNK * 256], f32, tag="et")
            etv = et[:].rearrange("p (j c) -> p j c", c=256)
            for gi, (g0, gn) in enumerate(GROUPS):
                trp = pst.tile([128, 512], f32, tag="tr")
                for jj in range(gn):
                    j = g0 + jj
                    nc.tensor.matmul(
                        out=r(trp[:, jj * 128:(jj + 1) * 128]),
                        lhsT=r(emb[:, j * 128:(j + 1) * 128]),
                        rhs=r(ident_t[:]),
                        is_transpose=True, start=True, stop=True,
                    )
                src = trp[:, :gn * 128].rearrange("p (j c) -> p j c", c=128)
                nc.vector.tensor_copy(out=etv[:, g0:g0 + gn, 0:128], in_=src)
                if gi < 4:
                    # square on ACT straight from PSUM
                    nc.scalar.activation(
                        out=etv[:, g0:g0 + gn, 128:256], in_=src, func=AF.Square,
                    )
                else:
                    # square on DVE from SBUF (after evict)
                    nc.vector.tensor_tensor(
                        out=etv[:, g0:g0 + gn, 128:256],
                        in0=etv[:, g0:g0 + gn, 0:128],
                        in1=etv[:, g0:g0 + gn, 0:128],
                        op=AL.mult,
                    )

            # ---- 4. seg-matmuls: [sumT | sumsqT] accumulate in [64, 256] ----
            seg = pseg.tile([D, 256], f32, tag="seg")
            for j in range(NCHUNK):
                nc.tensor.matmul(
                    out=seg[:],
                    lhsT=r(segf_t[:]),
                    rhs=r(et[:, j * 256:(j + 1) * 256]),
                    start=(j == 0), stop=(j == NCHUNK - 1),
                    skip_group_check=True,
                )

            # ---- summedT (+aug ones row) ----
            saug = sb.tile([D + 1, 128], f32, tag="saug")
            nc.vector.tensor_tensor(
                out=saug[0:D, :], in0=seg[:, 0:128], in1=numembT[:], op=AL.add,
            )
            nc.vector.tensor_copy(out=saug[D:D + 1, :], in_=onesrow_t[:])

            # ---- 5. u-matmuls ----
            # bias: one N=512 matmul with rhs = summedT_aug repeated 4x
            saug_rep = (
                saug[:].rearrange("p (o n) -> p o n", o=1)
                .to_broadcast([D + 1, 4, 128])
            )
            r_buf = sb.tile([128, NCHUNK * 128], bf16, tag="rbuf")
            for gi, (g0, gn) in enumerate(GROUPS):
                up = psu.tile([128, 512], f32, tag="u")
                if gn == 4:
                    nc.tensor.matmul(
                        out=up[:], lhsT=r(waug_t[:]), rhs=r(saug_rep),
                        start=True, stop=False, skip_group_check=True,
                    )
                    for pp in range(2):  # support pairs N=256
                        rhs = et[:].rearrange("p (j c) -> p j c", c=256)[
                            :, g0 + 2 * pp:g0 + 2 * pp + 2, 0:128
                        ]
                        nc.tensor.matmul(
                            out=up[:, pp * 256:(pp + 1) * 256],
                            lhsT=r(gw2_t[:]), rhs=r(rhs),
                            start=False, stop=True, skip_group_check=True,
                        )
                else:
                    nc.tensor.matmul(
                        out=up[:, 0:128], lhsT=r(waug_t[:]), rhs=r(saug[:]),
                        start=True, stop=False, skip_group_check=True,
                    )
                    nc.tensor.matmul(
                        out=up[:, 0:128], lhsT=r(gw2_t[:]),
                        rhs=r(et[:, g0 * 256:g0 * 256 + 128]),
                        start=False, stop=True, skip_group_check=True,
                    )
                # ---- 6. relu-evict to bf16 ----
                nc.scalar.activation(
                    out=r_buf[:, g0 * 128:(g0 + gn) * 128],
                    in_=up[:, :gn * 128], func=AF.Relu,
                )

            # num field u + relu
            unum = psm.tile([D, 128], f32, tag="small")
            nc.tensor.matmul(
                out=unum[:], lhsT=r(waug_t[:, 0:D]), rhs=r(saug[:]),
                start=True, stop=False, skip_group_check=True,
            )
            nc.tensor.matmul(
                out=unum[:], lhsT=r(gw2_t[0:D, 0:D]), rhs=r(numembT[:]),
                start=False, stop=True, skip_group_check=True,
            )
            rnum = sb.tile([D, 128], bf16, tag="rnum")
            nc.scalar.activation(out=rnum[:], in_=unum[:], func=AF.Relu)

            # ---- racc: g_preT = sum_f relu(u_f) ----
            gpre = pracc.tile([D, 128], f32, tag="gpre")
            for j in range(NCHUNK):
                nc.tensor.matmul(
                    out=gpre[:], lhsT=segb_t[:], rhs=r_buf[:, j * 128:(j + 1) * 128],
                    start=(j == 0), stop=False, skip_group_check=True,
                )
            nc.tensor.matmul(
                out=gpre[:], lhsT=i64b_t[:], rhs=rnum[:],
                start=False, stop=True, skip_group_check=True,
            )
            gpreT = sb.tile([D, 128], f32, tag="gpreT")
            nc.scalar.copy(out=gpreT[:], in_=gpre[:])

            # ---- local branch: lT = summedT^2 - sumsqT ----
            lT = sb.tile([D, 128], f32, tag="lT")
            nc.vector.tensor_tensor(
                out=lT[:], in0=saug[0:D, :], in1=saug[0:D, :], op=AL.mult,
            )
            nc.vector.tensor_tensor(
                out=lT[:], in0=lT[:], in1=seg[:, 128:256], op=AL.subtract,
            )

            # ---- MLPs (transposed) ----
            h1p = psm.tile([D, 128], f32, tag="small")
            nc.tensor.matmul(out=h1p[:], lhsT=g0_t[:], rhs=gpreT[:],
                             start=True, stop=True)
            h1aug = sb.tile([D + 1, 128], f32, tag="h1aug")
            nc.scalar.activation(out=h1aug[0:D, :], in_=h1p[:], func=AF.Relu,
                                 bias=gb0_c)
            nc.vector.tensor_copy(out=h1aug[D:D + 1, :], in_=onesrow_t[:])

            l1p = psm.tile([D, 128], f32, tag="small")
            nc.tensor.matmul(out=l1p[:], lhsT=l0_t[:], rhs=lT[:],
                             start=True, stop=True)
            l1aug = sb.tile([D + 1, 128], f32, tag="l1aug")
            nc.scalar.activation(out=l1aug[0:D, :], in_=l1p[:], func=AF.Relu,
                                 bias=lb0_c)
            nc.vector.tensor_copy(out=l1aug[D:D + 1, :], in_=onesrow_t[:])

            outp = psm.tile([D, 128], f32, tag="small")
            nc.tensor.matmul(out=outp[:], lhsT=g1_t[:], rhs=h1aug[:],
                             start=True, stop=False, skip_group_check=True)
            nc.tensor.matmul(out=outp[:], lhsT=l1_t[:], rhs=l1aug[:],
                             start=False, stop=True, skip_group_check=True)
            outT = sb.tile([D, 128], f32, tag="outT")
            nc.scalar.copy(out=outT[:], in_=outp[:])

            # ---- transpose back to [128, 64] and store ----
            finp = psm.tile([128, D], f32, tag="small")
            nc.tensor.matmul(out=finp[:], lhsT=outT[:], rhs=i64f_t[:],
                             is_transpose=True, start=True, stop=True)
            orow = sb.tile([128, D], f32, tag="orow")
            nc.vector.tensor_copy(out=orow[:], in_=finp[:])
            nc.sync.dma_start(out_d[blk * BLK:(blk + 1) * BLK, :], orow[:])

        if rep_cm is not None:
            rep_cm.__exit__(None, None, None)

    return nc


def _make_consts(embed_table, num_W, num_b, ga_W, ga_b, gW, gb, lW, lb):
    """Host-side constant prep. Returns dict of name -> np.ndarray."""
    f = np.float32
    ga_W = ga_W.astype(f)
    ident128 = np.eye(128, dtype=f)
    i64 = np.eye(D, dtype=f)
    seg = np.vstack([i64, i64]).astype(f)           # [128, 64]
    waug = np.zeros((D + 1, 128), f)                # bias matmul lhsT
    waug[:D, :D] = ga_W / CD
    waug[:D, D:] = ga_W / CD
    waug[D, :D] = ga_b
    waug[D, D:] = ga_b
    gw2 = np.zeros((128, 128), f)                   # blockdiag support lhsT
    gw2[:D, :D] = ga_W * (PROBE / CD)
    gw2[D:, D:] = ga_W * (PROBE / CD)
    g0T = (gW[0].astype(f) / NF).T.copy()           # fold 1/51 mean
    g1aug = np.zeros((D + 1, D), f)
    g1aug[:D] = ALPHA * gW[1].astype(f).T
    g1aug[D] = ALPHA * gb[1].astype(f)
    l0T = (0.5 * lW[0].astype(f)).T.copy()          # fold FM 0.5
    l1aug = np.zeros((D + 1, D), f)
    l1aug[:D] = (1.0 - ALPHA) * lW[1].astype(f).T
    l1aug[D] = (1.0 - ALPHA) * lb[1].astype(f)
    cols = np.stack(
        [num_W[:, 0].astype(f), num_b.astype(f), gb[0].astype(f), lb[0].astype(f)],
        axis=1,
    ).copy()                                        # [64, 4]
    return {
        "table": np.ascontiguousarray(embed_table.astype(f)),
        "ident128": ident128,
        "seg_f": seg,
        "seg_b": seg,          # cast to bf16 at map build
        "i64_b": i64,          # cast to bf16 at map build
        "i64_f": i64,
        "waug": waug,
        "gw2": gw2,
        "g0T": g0T,
        "g1aug": g1aug,
        "l0T": l0T,
        "l1aug": l1aug,
        "cols": cols,
        "ones164": np.ones((1, D), f),
    }


def make_in_maps(inputs):
    """Shard FULL inputs into per-core input maps (host-side prep)."""
    import ml_dtypes

    consts = _make_consts(
        inputs["embed_table"], inputs["num_W"], inputs["num_b"],
        inputs["ga_W"], inputs["ga_b"], inputs["gW"], inputs["gb"],
        inputs["lW"], inputs["lb"],
    )
    bf = ml_dtypes.bfloat16
    cmap = {
        k: (v.astype(bf) if k in ("seg_b", "i64_b") else v)
        for k, v in consts.items()
    }
    idx32 = np.ascontiguousarray(np.asarray(inputs["cat_indices"]).astype(np.int32))
    numf = np.ascontiguousarray(
        np.asarray(inputs["num_features"]).astype(np.float32)
    )
    in_maps = []
    for c in range(NCORES):
        m = dict(cmap)
        m["cat_idx"] = idx32[c * BS:(c + 1) * BS]
        m["numf"] = numf[c * BS:(c + 1) * BS]
        in_maps.append(m)
    return in_maps


def kernel(cat_indices, num_features, embed_table, num_W, num_b,
           ga_W, ga_b, gW, gb, lW, lb):
    from concourse.bass_utils import run_bass_kernel_spmd

    if "nc" not in _CACHE:
        print("[kernel] building bass module...", flush=True)
        nc = _build()
        print("[kernel] finalizing...", flush=True)
        nc.finalize()
        _CACHE["nc"] = nc
        print("[kernel] build done", flush=True)
    nc = _CACHE["nc"]

    in_maps = make_in_maps(dict(
        cat_indices=cat_indices, num_features=num_features,
        embed_table=embed_table, num_W=num_W, num_b=num_b,
        ga_W=ga_W, ga_b=ga_b, gW=gW, gb=gb, lW=lW, lb=lb,
    ))

    print("[kernel] launching spmd run...", flush=True)
    res = run_bass_kernel_spmd(nc, in_maps, list(range(NCORES)))
    print("[kernel] run complete", flush=True)
    outs = [res.results[c]["out"] for c in range(NCORES)]
    return np.concatenate(outs, axis=0).astype(np.float32)



# revision 6
# speedup vs baseline: 1.3904x; 1.3904x over previous
"""Trainium2 Bass kernel for nn_CatEmbedder (gnn_message_passing).

Takes FULL inputs, shards batch B=32768 across 8 NeuronCores (4096 each),
replicates the embedding table + weights, runs an SPMD Bass kernel, and
concatenates + un-permutes the per-core outputs.

Gather strategy (the bottleneck): instead of 50 per-field indirect DMAs
per 128-sample block (~1us Pool-engine SWDGE time each), use 4 dma_gather
instructions per block on SWDGE queues 0-3 (concurrent Q7 cpu pairs).
dma_gather takes int16 indices, so the 100k-row table is split into 4
range-buckets of 25000 rows; each sample's 50 indices are pre-sorted so
its bucket-q rows occupy a contiguous run of its slots. Per-sample counts
vary, so each bucket region is padded to a per-slot capacity with
pointers to a zero row (prepended per bucket in a rebuilt table); zero
rows are sum/sumsq-neutral and their relu(base) contribution is
subtracted exactly via a per-sample pad-count correction.

Samples are globally sorted by bucket-count profile and dealt round-robin
to (core, slot) so all 8 cores share one NEFF with tight per-slot
capacities.

Per-core pipeline per block (ST = padded stripes, ~56-62):
  1. 4x dma_gather (queues 0-3, single_packet=False) -> emb [128, ST*64]
  2. PE transposes of [128,128] chunks -> PSUM -> SBUF (et)
  3. squares (ACT/DVE split) into interleaved sq chunks
  4. PE seg-matmuls: field-sum + field-sum-of-squares -> [64,256] PSUM
  5. PE u-matmuls: u_f = (S + PROBE*support_f)/c + ga_b (bias via aug row)
  6. ACT relu-evict (bf16) -> PE accumulates sum_f relu(u_f); subtract
     npad * relu(base) pad correction
  7. transposed MLPs for global/local branches, combine, transpose, store
"""

import os
import sys
import numpy as np

sys.path.insert(0, "/opt/trn_rl_repo")

# ---- problem constants (hardcoded per the contract) ----
B, F, D, NCT = 32768, 50, 64, 100000
PROBE, ALPHA = 39.0, 0.5
NF = F + 1              # 51 fields
CD = NF + PROBE         # 90.0
NCORES = 8
BS = B // NCORES        # 4096 samples per core
BLK = 128
NBLK = BS // BLK        # 32 blocks (slots) per core
NBUCK = 4
BUCK = NCT // NBUCK     # 25000 rows per bucket
TROW = BUCK + 1         # bucket stride in table2 (incl. zero row)

_CACHE = {}


def _host_prep(cat_indices, num_features):
    """Sort/cluster/bucket the indices. Returns per-core tensors + meta."""
    idx = np.asarray(cat_indices).astype(np.int64)
    numf = np.asarray(num_features).astype(np.float32)

    idx_sorted = np.sort(idx, axis=1)                        # [B, 50]
    # bucket counts per sample
    c = np.stack([
        (idx_sorted < BUCK).sum(1),
        ((idx_sorted >= BUCK) & (idx_sorted < 2 * BUCK)).sum(1),
        ((idx_sorted >= 2 * BUCK) & (idx_sorted < 3 * BUCK)).sum(1),
        (idx_sorted >= 3 * BUCK).sum(1),
    ], axis=1)                                               # [B, 4]
    o = np.concatenate([np.zeros((B, 1), np.int64),
                        np.cumsum(c, axis=1)], axis=1)       # [B, 5]

    # cluster sample profiles via recursive bisection so each slot's 1024
    # samples (8 cores x 128) have tight per-bucket count maxima
    def bisect(ordr, keys, splits):
        if not splits:
            return [ordr]
        o = ordr[np.argsort(c[ordr, keys[0]], kind="stable")]
        out = []
        for ch in np.array_split(o, splits[0]):
            out += bisect(ch, keys[1:], splits[1:])
        return out

    order = np.concatenate(
        bisect(np.arange(B), (3, 0, 1, 2), (2, 4, 2, 2))
    )
    # blocks of 128 consecutive samples; slot s gets blocks s*8+core
    blocks = order.reshape(B // BLK, BLK)                    # [256, 128]
    cb = np.stack([c[blocks, q].max(axis=1) for q in range(NBUCK)], axis=1)

    caps = np.zeros((NBLK, NBUCK), np.int64)
    for s in range(NBLK):
        caps[s] = cb[s * NCORES:(s + 1) * NCORES].max(axis=0)
        if caps[s].sum() % 2:
            caps[s, 0] += 1                                  # even stripes
    st = caps.sum(axis=1)                                    # [NBLK]
    n16 = caps * BLK // 16                                   # [NBLK, NBUCK]
    n16max = int(n16.max())

    idx16 = np.zeros((NCORES, NBLK, 128, n16max), np.int16)
    npad = np.zeros((NCORES, NBLK, 64, BLK), np.float32)
    numf_r = np.zeros((NCORES, NBLK, 1, BLK), np.float32)
    rows_all = np.zeros((NCORES, BS), np.int64)

    for s in range(NBLK):
        for core in range(NCORES):
            samp = blocks[s * NCORES + core]                 # [128] sample ids
            rows_all[core, s * BLK:(s + 1) * BLK] = samp
            numf_r[core, s, 0] = numf[samp]
            npad[core, s, :, :] = (caps[s][None, :] - c[samp]).sum(axis=1)[None, :]
            for q in range(NBUCK):
                C = int(caps[s, q])
                if C == 0:
                    continue
                # vals[p, k] = sorted idx (local+1) or 0 pad
                k = np.arange(C)[None, :]                    # [1, C]
                cpq = c[samp, q][:, None]                    # [128, 1]
                opq = o[samp, q][:, None]
                take = np.clip(opq + k, 0, F - 1)
                v = idx_sorted[samp[:, None], take] - q * BUCK + 1
                vals = np.where(k < cpq, v, 0).astype(np.int16)  # [128, C]
                flat = vals.T.ravel()                        # j = k*128+p
                wrapped = flat.reshape(-1, 16).T             # [16, C*8]
                band = np.tile(wrapped, (2, 1))              # [32, C*8]
                idx16[core, s, 32 * q:32 * q + 32, :C * 8] = band

    meta = {
        "caps": caps, "st": st, "n16max": n16max, "rows_all": rows_all,
    }
    return idx16, npad, numf_r, meta


def _build(caps, st, n16max):
    import concourse.bass as bass
    import concourse.mybir as mybir
    import concourse.tile as tile
    from concourse import bacc
    from contextlib import ExitStack

    f32 = mybir.dt.float32
    bf16 = mybir.dt.bfloat16
    i16 = mybir.dt.int16
    AL = mybir.AluOpType
    AF = mybir.ActivationFunctionType

    STMAX = int(max(st))
    NCHMAX = STMAX // 2

    nc = bacc.Bacc(None, num_swdge_queues=4, dynamic_dma_scratch_size=32768)

    idx_d = nc.declare_dram_parameter("idx16", [NBLK, 128, n16max], i16,
                                      isOutput=False)
    npad_d = nc.declare_dram_parameter("npad", [NBLK, D, BLK], f32,
                                       isOutput=False)
    numf_d = nc.declare_dram_parameter("numfr", [NBLK, 1, BLK], f32,
                                       isOutput=False)
    table_d = nc.declare_dram_parameter("table2", [NBUCK * TROW, D], f32,
                                        isOutput=False)
    ident_d = nc.declare_dram_parameter("ident128", [128, 128], f32,
                                        isOutput=False)
    segf_d = nc.declare_dram_parameter("seg_f", [128, D], f32, isOutput=False)
    segb_d = nc.declare_dram_parameter("seg_b", [128, D], bf16, isOutput=False)
    i64b_d = nc.declare_dram_parameter("i64_b", [D, D], bf16, isOutput=False)
    i64f_d = nc.declare_dram_parameter("i64_f", [D, D], f32, isOutput=False)
    waug_d = nc.declare_dram_parameter("waug", [D + 1, 128], f32,
                                       isOutput=False)
    gw2_d = nc.declare_dram_parameter("gw2", [128, 128], f32, isOutput=False)
    g0_d = nc.declare_dram_parameter("g0T", [D, D], f32, isOutput=False)
    g1_d = nc.declare_dram_parameter("g1aug", [D + 1, D], f32, isOutput=False)
    l0_d = nc.declare_dram_parameter("l0T", [D, D], f32, isOutput=False)
    l1_d = nc.declare_dram_parameter("l1aug", [D + 1, D], f32, isOutput=False)
    cols_d = nc.declare_dram_parameter("cols", [D, 4], f32, isOutput=False)
    ones_d = nc.declare_dram_parameter("ones164", [1, D], f32, isOutput=False)
    out_d = nc.declare_dram_parameter("out", [BS, D], f32, isOutput=True)

    with tile.TileContext(nc) as tc, ExitStack() as ctx:
        const = ctx.enter_context(tc.tile_pool(name="const", bufs=1))
        sb = ctx.enter_context(tc.tile_pool(name="sb", bufs=2))
        pst = ctx.enter_context(tc.tile_pool(name="pst", bufs=2, space="PSUM"))
        psu = ctx.enter_context(tc.tile_pool(name="psu", bufs=2, space="PSUM"))
        pseg = ctx.enter_context(tc.tile_pool(name="pseg", bufs=1,
                                              space="PSUM"))
        pracc = ctx.enter_context(tc.tile_pool(name="pracc", bufs=1,
                                               space="PSUM"))
        psm = ctx.enter_context(tc.tile_pool(name="psm", bufs=1, space="PSUM"))

        ident_t = const.tile([128, 128], f32)
        nc.sync.dma_start(ident_t[:], ident_d[:])
        segf_t = const.tile([128, D], f32)
        nc.sync.dma_start(segf_t[:], segf_d[:])
        segb_t = const.tile([128, D], bf16)
        nc.sync.dma_start(segb_t[:], segb_d[:])
        i64b_t = const.tile([D, D], bf16)
        nc.sync.dma_start(i64b_t[:], i64b_d[:])
        i64f_t = const.tile([D, D], f32)
        nc.sync.dma_start(i64f_t[:], i64f_d[:])
        waug_t = const.tile([D + 1, 128], f32)
        nc.sync.dma_start(waug_t[:], waug_d[:])
        gw2_t = const.tile([128, 128], f32)
        nc.sync.dma_start(gw2_t[:], gw2_d[:])
        g0_t = const.tile([D, D], f32)
        nc.sync.dma_start(g0_t[:], g0_d[:])
        g1_t = const.tile([D + 1, D], f32)
        nc.sync.dma_start(g1_t[:], g1_d[:])
        l0_t = const.tile([D, D], f32)
        nc.sync.dma_start(l0_t[:], l0_d[:])
        l1_t = const.tile([D + 1, D], f32)
        nc.sync.dma_start(l1_t[:], l1_d[:])
        cols_t = const.tile([D, 4], f32)
        nc.sync.dma_start(cols_t[:], cols_d[:])
        ones_t = const.tile([1, D], f32)
        nc.sync.dma_start(ones_t[:], ones_d[:])
        onesrow_t = const.tile([1, 128], f32)
        nc.vector.memset(onesrow_t[:], 1.0)

        numw_c = cols_t[:, 0:1]
        numb_c = cols_t[:, 1:2]
        gb0_c = cols_t[:, 2:3]
        lb0_c = cols_t[:, 3:4]

        for blk in range(NBLK):
            ST = int(st[blk])
            NCHUNK = ST // 2
            GROUPS = []
            g0 = 0
            while g0 < NCHUNK:
                gn = min(4, NCHUNK - g0)
                GROUPS.append((g0, gn))
                g0 += gn

            idx_t = sb.tile([128, n16max], i16, tag="idx")
            nc.sync.dma_start(idx_t[:], idx_d[blk])
            numf_t = sb.tile([1, BLK], f32, tag="numf")
            nc.sync.dma_start(numf_t[:], numf_d[blk])
            npad64 = sb.tile([D, BLK], f32, tag="npad64")
            nc.sync.dma_start(npad64[:, :], npad_d[blk])

            # ---- 1. bucketed gathers on queues 0-3 ----
            emb = sb.tile([128, STMAX * D], f32, tag="emb")
            off = 0
            for q in range(NBUCK):
                C = int(caps[blk, q])
                if C == 0:
                    continue
                nc.gpsimd.dma_gather(
                    out_ap=emb[:, off * D:(off + C) * D].rearrange(
                        "p (s d) -> p s d", d=D
                    ),
                    in_ap=table_d[q * TROW:(q + 1) * TROW, :],
                    idxs_ap=idx_t[:, 0:C * 8],
                    num_idxs=C * BLK,
                    num_idxs_reg=C * BLK,
                    elem_size=D,
                    single_packet=False,
                    queue_num=q,
                )
                off += C

            # ---- numeric-field embedding (transposed): [64, 128] ----
            nrep = psm.tile([D, 128], f32, tag="small")
            nc.tensor.matmul(out=nrep[:], lhsT=ones_t[:], rhs=numf_t[:],
                             start=True, stop=True)
            numembT = sb.tile([D, 128], f32, tag="numembT")
            nc.scalar.activation(out=numembT[:], in_=nrep[:], func=AF.Identity,
                                 bias=numb_c, scale=numw_c)

            # ---- 2. transposes + evict; 3. squares ----
            et = sb.tile([128, NCHMAX * 256], f32, tag="et")
            etv = et[:].rearrange("p (j c) -> p j c", c=256)
            for gi, (gg0, gn) in enumerate(GROUPS):
                trp = pst.tile([128, 512], f32, tag="tr")
                for jj in range(gn):
                    j = gg0 + jj
                    nc.tensor.matmul(
                        out=trp[:, jj * 128:(jj + 1) * 128],
                        lhsT=emb[:, j * 128:(j + 1) * 128],
                        rhs=ident_t[:],
                        is_transpose=True, start=True, stop=True,
                    )
                src = trp[:, :gn * 128].rearrange("p (j c) -> p j c", c=128)
                nc.vector.tensor_copy(out=etv[:, gg0:gg0 + gn, 0:128], in_=src)
                if gi % 2 == 0:
                    nc.scalar.activation(
                        out=etv[:, gg0:gg0 + gn, 128:256], in_=src,
                        func=AF.Square,
                    )
                else:
                    nc.vector.tensor_tensor(
                        out=etv[:, gg0:gg0 + gn, 128:256],
                        in0=etv[:, gg0:gg0 + gn, 0:128],
                        in1=etv[:, gg0:gg0 + gn, 0:128],
                        op=AL.mult,
                    )

            # ---- 4. seg-matmuls: [sumT | sumsqT] -> [64, 256] PSUM ----
            seg = pseg.tile([D, 256], f32, tag="seg")
            for j in range(NCHUNK):
                nc.tensor.matmul(
                    out=seg[:], lhsT=segf_t[:],
                    rhs=et[:, j * 256:(j + 1) * 256],
                    start=(j == 0), stop=(j == NCHUNK - 1),
                    skip_group_check=True,
                )

            # ---- summedT (+aug ones row) ----
            saug = sb.tile([D + 1, 128], f32, tag="saug")
            nc.vector.tensor_tensor(out=saug[0:D, :], in0=seg[:, 0:128],
                                    in1=numembT[:], op=AL.add)
            nc.vector.tensor_copy(out=saug[D:D + 1, :], in_=onesrow_t[:])

            # ---- 5. u-matmuls ----
            r_buf = sb.tile([128, NCHMAX * 128], bf16, tag="rbuf")
            for gi, (gg0, gn) in enumerate(GROUPS):
                up = psu.tile([128, 512], f32, tag="u")
                saug_rep = (
                    saug[:].rearrange("p (o n) -> p o n", o=1)
                    .to_broadcast([D + 1, gn, 128])
                )
                nc.tensor.matmul(
                    out=up[:, 0:gn * 128], lhsT=waug_t[:], rhs=saug_rep,
                    start=True, stop=False, skip_group_check=True,
                )
                pp = 0
                while pp < gn:
                    pn = min(2, gn - pp)
                    rhs = et[:].rearrange("p (j c) -> p j c", c=256)[
                        :, gg0 + pp:gg0 + pp + pn, 0:128
                    ]
                    nc.tensor.matmul(
                        out=up[:, pp * 128:(pp + pn) * 128],
                        lhsT=gw2_t[:], rhs=rhs,
                        start=False, stop=True, skip_group_check=True,
                    )
                    pp += pn
                nc.scalar.activation(
                    out=r_buf[:, gg0 * 128:(gg0 + gn) * 128],
                    in_=up[:, :gn * 128], func=AF.Relu,
                )

            # num field u + relu; also relu(base) for pad correction
            unum = psm.tile([D, 128], f32, tag="small")
            nc.tensor.matmul(out=unum[:], lhsT=waug_t[:, 0:D], rhs=saug[:],
                             start=True, stop=False, skip_group_check=True)
            nc.tensor.matmul(out=unum[:], lhsT=gw2_t[0:D, 0:D], rhs=numembT[:],
                             start=False, stop=True, skip_group_check=True)
            rnum = sb.tile([D, 128], bf16, tag="rnum")
            nc.scalar.activation(out=rnum[:], in_=unum[:], func=AF.Relu)

            ubase = psm.tile([D, 128], f32, tag="small")
            nc.tensor.matmul(out=ubase[:], lhsT=waug_t[:, 0:D], rhs=saug[:],
                             start=True, stop=True)
            rbase = sb.tile([D, 128], f32, tag="rbase")
            nc.scalar.activation(out=rbase[:], in_=ubase[:], func=AF.Relu)

            # ---- racc: g_preT = sum_f relu(u_f) ----
            gpre = pracc.tile([D, 128], f32, tag="gpre")
            for j in range(NCHUNK):
                nc.tensor.matmul(
                    out=gpre[:], lhsT=segb_t[:],
                    rhs=r_buf[:, j * 128:(j + 1) * 128],
                    start=(j == 0), stop=False, skip_group_check=True,
                )
            nc.tensor.matmul(out=gpre[:], lhsT=i64b_t[:], rhs=rnum[:],
                             start=False, stop=True, skip_group_check=True)
            gpreT = sb.tile([D, 128], f32, tag="gpreT")
            nc.scalar.copy(out=gpreT[:], in_=gpre[:])
            # pad correction: gpreT -= npad64 * relu(base)
            corr = sb.tile([D, 128], f32, tag="corr")
            nc.vector.tensor_tensor(out=corr[:], in0=rbase[:], in1=npad64[:, :],
                                    op=AL.mult)
            nc.vector.tensor_tensor(out=gpreT[:], in0=gpreT[:], in1=corr[:],
                                    op=AL.subtract)

            # ---- local branch: lT = summedT^2 - sumsqT ----
            lT = sb.tile([D, 128], f32, tag="lT")
            nc.vector.tensor_tensor(out=lT[:], in0=saug[0:D, :],
                                    in1=saug[0:D, :], op=AL.mult)
            nc.vector.tensor_tensor(out=lT[:], in0=lT[:], in1=seg[:, 128:256],
                                    op=AL.subtract)

            # ---- MLPs (transposed) ----
            h1p = psm.tile([D, 128], f32, tag="small")
            nc.tensor.matmul(out=h1p[:], lhsT=g0_t[:], rhs=gpreT[:],
                             start=True, stop=True)
            h1aug = sb.tile([D + 1, 128], f32, tag="h1aug")
            nc.scalar.activation(out=h1aug[0:D, :], in_=h1p[:], func=AF.Relu,
                                 bias=gb0_c)
            nc.vector.tensor_copy(out=h1aug[D:D + 1, :], in_=onesrow_t[:])

            l1p = psm.tile([D, 128], f32, tag="small")
            nc.tensor.matmul(out=l1p[:], lhsT=l0_t[:], rhs=lT[:],
                             start=True, stop=True)
            l1aug = sb.tile([D + 1, 128], f32, tag="l1aug")
            nc.scalar.activation(out=l1aug[0:D, :], in_=l1p[:], func=AF.Relu,
                                 bias=lb0_c)
            nc.vector.tensor_copy(out=l1aug[D:D + 1, :], in_=onesrow_t[:])

            outp = psm.tile([D, 128], f32, tag="small")
            nc.tensor.matmul(out=outp[:], lhsT=g1_t[:], rhs=h1aug[:],
                             start=True, stop=False, skip_group_check=True)
            nc.tensor.matmul(out=outp[:], lhsT=l1_t[:], rhs=l1aug[:],
                             start=False, stop=True, skip_group_check=True)
            outT = sb.tile([D, 128], f32, tag="outT")
            nc.scalar.copy(out=outT[:], in_=outp[:])

            finp = psm.tile([128, D], f32, tag="smallo")
            nc.tensor.matmul(out=finp[:], lhsT=outT[:], rhs=i64f_t[:],
                             is_transpose=True, start=True, stop=True)
            orow = sb.tile([128, D], f32, tag="orow")
            nc.vector.tensor_copy(out=orow[:], in_=finp[:])
            nc.sync.dma_start(out_d[blk * BLK:(blk + 1) * BLK, :], orow[:])

    return nc


def _make_consts(embed_table, num_W, num_b, ga_W, ga_b, gW, gb, lW, lb):
    """Host-side constant prep. Returns dict of name -> np.ndarray."""
    f = np.float32
    ga_W = ga_W.astype(f)
    table = np.asarray(embed_table).astype(f)
    table2 = np.zeros((NBUCK * TROW, D), f)
    for q in range(NBUCK):
        table2[q * TROW] = 0.0
        table2[q * TROW + 1:(q + 1) * TROW] = table[q * BUCK:(q + 1) * BUCK]
    ident128 = np.eye(128, dtype=f)
    i64 = np.eye(D, dtype=f)
    seg = np.vstack([i64, i64]).astype(f)           # [128, 64]
    waug = np.zeros((D + 1, 128), f)                # bias matmul lhsT
    waug[:D, :D] = ga_W / CD
    waug[:D, D:] = ga_W / CD
    waug[D, :D] = ga_b
    waug[D, D:] = ga_b
    gw2 = np.zeros((128, 128), f)                   # blockdiag support lhsT
    gw2[:D, :D] = ga_W * (PROBE / CD)
    gw2[D:, D:] = ga_W * (PROBE / CD)
    g0T = (gW[0].astype(f) / NF).T.copy()           # fold 1/51 mean
    g1aug = np.zeros((D + 1, D), f)
    g1aug[:D] = ALPHA * gW[1].astype(f).T
    g1aug[D] = ALPHA * gb[1].astype(f)
    l0T = (0.5 * lW[0].astype(f)).T.copy()          # fold FM 0.5
    l1aug = np.zeros((D + 1, D), f)
    l1aug[:D] = (1.0 - ALPHA) * lW[1].astype(f).T
    l1aug[D] = (1.0 - ALPHA) * lb[1].astype(f)
    cols = np.stack(
        [num_W[:, 0].astype(f), num_b.astype(f), gb[0].astype(f),
         lb[0].astype(f)], axis=1,
    ).copy()
    return {
        "table2": table2,
        "ident128": ident128,
        "seg_f": seg,
        "seg_b": seg,
        "i64_b": i64,
        "i64_f": i64,
        "waug": waug,
        "gw2": gw2,
        "g0T": g0T,
        "g1aug": g1aug,
        "l0T": l0T,
        "l1aug": l1aug,
        "cols": cols,
        "ones164": np.ones((1, D), f),
    }


def prepare(inputs):
    """Build (cached) nc + per-core in_maps + meta from FULL inputs."""
    import ml_dtypes

    idx16, npad, numf_r, meta = _host_prep(
        inputs["cat_indices"], inputs["num_features"]
    )
    consts = _make_consts(
        inputs["embed_table"], inputs["num_W"], inputs["num_b"],
        inputs["ga_W"], inputs["ga_b"], inputs["gW"], inputs["gb"],
        inputs["lW"], inputs["lb"],
    )
    bf = ml_dtypes.bfloat16
    cmap = {
        k: (v.astype(bf) if k in ("seg_b", "i64_b") else v)
        for k, v in consts.items()
    }

    key = (tuple(meta["caps"].ravel().tolist()), meta["n16max"])
    if _CACHE.get("key") != key:
        print("[kernel] building bass module...", flush=True)
        nc = _build(meta["caps"], meta["st"], meta["n16max"])
        print("[kernel] finalizing...", flush=True)
        nc.finalize()
        _CACHE.update({"nc": nc, "key": key})
        print("[kernel] build done", flush=True)
    nc = _CACHE["nc"]

    in_maps = []
    for c in range(NCORES):
        m = dict(cmap)
        m["idx16"] = np.ascontiguousarray(idx16[c])
        m["npad"] = np.ascontiguousarray(npad[c])
        m["numfr"] = np.ascontiguousarray(numf_r[c])
        in_maps.append(m)
    return nc, in_maps, meta


def kernel(cat_indices, num_features, embed_table, num_W, num_b,
           ga_W, ga_b, gW, gb, lW, lb):
    from concourse.bass_utils import run_bass_kernel_spmd

    nc, in_maps, meta = prepare(dict(
        cat_indices=cat_indices, num_features=num_features,
        embed_table=embed_table, num_W=num_W, num_b=num_b,
        ga_W=ga_W, ga_b=ga_b, gW=gW, gb=gb, lW=lW, lb=lb,
    ))

    print("[kernel] launching spmd run...", flush=True)
    res = run_bass_kernel_spmd(nc, in_maps, list(range(NCORES)))
    print("[kernel] run complete", flush=True)
    out = np.empty((B, D), np.float32)
    for c in range(NCORES):
        out[meta["rows_all"][c]] = res.results[c]["out"]
    return out


# revision 9
# speedup vs baseline: 1.3995x; 1.0065x over previous
"""Trainium2 Bass kernel for nn_CatEmbedder (gnn_message_passing).

Takes FULL inputs, shards batch B=32768 across 8 NeuronCores (4096 each),
replicates the embedding table + weights, runs an SPMD Bass kernel, and
concatenates + un-permutes the per-core outputs.

Gather strategy (the bottleneck): instead of 50 per-field indirect DMAs
per 128-sample block (~1us Pool-engine SWDGE time each), use 4 dma_gather
instructions per block on SWDGE queues 0-3 (concurrent Q7 cpu pairs).
dma_gather takes int16 indices, so the 100k-row table is split into 4
range-buckets of 25000 rows; each sample's 50 indices are pre-sorted so
its bucket-q rows occupy a contiguous run of its slots. Per-sample counts
vary, so each bucket region is padded to a per-slot capacity with
pointers to a zero row (prepended per bucket in a rebuilt table); zero
rows are sum/sumsq-neutral and their relu(base) contribution is
subtracted exactly via a per-sample pad-count correction.

Samples are globally sorted by bucket-count profile and dealt round-robin
to (core, slot) so all 8 cores share one NEFF with tight per-slot
capacities.

Per-core pipeline per block (ST = padded stripes, ~56-62):
  1. 4x dma_gather (queues 0-3, single_packet=False) -> emb [128, ST*64]
  2. PE transposes of [128,128] chunks -> PSUM -> SBUF (et)
  3. squares (ACT/DVE split) into interleaved sq chunks
  4. PE seg-matmuls: field-sum + field-sum-of-squares -> [64,256] PSUM
  5. PE u-matmuls: u_f = (S + PROBE*support_f)/c + ga_b (bias via aug row)
  6. ACT relu-evict (bf16) -> PE accumulates sum_f relu(u_f); subtract
     npad * relu(base) pad correction
  7. transposed MLPs for global/local branches, combine, transpose, store
"""

import os
import sys
import numpy as np

sys.path.insert(0, "/opt/trn_rl_repo")

# ---- problem constants (hardcoded per the contract) ----
B, F, D, NCT = 32768, 50, 64, 100000
PROBE, ALPHA = 39.0, 0.5
NF = F + 1              # 51 fields
CD = NF + PROBE         # 90.0
NCORES = 8
BS = B // NCORES        # 4096 samples per core
BLK = 128
NBLK = BS // BLK        # 32 blocks (slots) per core
NBUCK = 4
BUCK = NCT // NBUCK     # 25000 rows per bucket
TROW = BUCK + 1         # bucket stride in table2 (incl. zero row)

_CACHE = {}


def _host_prep(cat_indices, num_features):
    """Sort/cluster/bucket the indices. Returns per-core tensors + meta."""
    idx = np.asarray(cat_indices).astype(np.int64)
    numf = np.asarray(num_features).astype(np.float32)

    idx_sorted = np.sort(idx, axis=1)                        # [B, 50]
    # bucket counts per sample
    c = np.stack([
        (idx_sorted < BUCK).sum(1),
        ((idx_sorted >= BUCK) & (idx_sorted < 2 * BUCK)).sum(1),
        ((idx_sorted >= 2 * BUCK) & (idx_sorted < 3 * BUCK)).sum(1),
        (idx_sorted >= 3 * BUCK).sum(1),
    ], axis=1)                                               # [B, 4]
    o = np.concatenate([np.zeros((B, 1), np.int64),
                        np.cumsum(c, axis=1)], axis=1)       # [B, 5]

    # cluster sample profiles via recursive bisection so each slot's 1024
    # samples (8 cores x 128) have tight per-bucket count maxima
    def bisect(ordr, keys, splits):
        if not splits:
            return [ordr]
        o = ordr[np.argsort(c[ordr, keys[0]], kind="stable")]
        out = []
        for ch in np.array_split(o, splits[0]):
            out += bisect(ch, keys[1:], splits[1:])
        return out

    order = np.concatenate(
        bisect(np.arange(B), (3, 0, 1, 2), (2, 4, 2, 2))
    )
    # blocks of 128 consecutive samples; slot s gets blocks s*8+core
    blocks = order.reshape(B // BLK, BLK)                    # [256, 128]
    cb = np.stack([c[blocks, q].max(axis=1) for q in range(NBUCK)], axis=1)

    caps = np.zeros((NBLK, NBUCK), np.int64)
    for s in range(NBLK):
        caps[s] = cb[s * NCORES:(s + 1) * NCORES].max(axis=0)
        if caps[s].sum() % 2:
            caps[s, 0] += 1                                  # even stripes
    st = caps.sum(axis=1)                                    # [NBLK]
    n16 = caps * BLK // 16                                   # [NBLK, NBUCK]
    n16max = int(n16.max())

    idx16 = np.zeros((NCORES, NBLK, 128, n16max), np.int16)
    npad = np.zeros((NCORES, NBLK, 64, BLK), np.float32)
    numf_r = np.zeros((NCORES, NBLK, 1, BLK), np.float32)
    rows_all = np.zeros((NCORES, BS), np.int64)

    for s in range(NBLK):
        for core in range(NCORES):
            samp = blocks[s * NCORES + core]                 # [128] sample ids
            rows_all[core, s * BLK:(s + 1) * BLK] = samp
            numf_r[core, s, 0] = numf[samp]
            npad[core, s, :, :] = (caps[s][None, :] - c[samp]).sum(axis=1)[None, :]
            for q in range(NBUCK):
                C = int(caps[s, q])
                if C == 0:
                    continue
                # vals[p, k] = sorted idx (local+1) or 0 pad
                k = np.arange(C)[None, :]                    # [1, C]
                cpq = c[samp, q][:, None]                    # [128, 1]
                opq = o[samp, q][:, None]
                take = np.clip(opq + k, 0, F - 1)
                v = idx_sorted[samp[:, None], take] - q * BUCK + 1
                vals = np.where(k < cpq, v, 0).astype(np.int16)  # [128, C]
                flat = vals.T.ravel()                        # j = k*128+p
                wrapped = flat.reshape(-1, 16).T             # [16, C*8]
                band = np.tile(wrapped, (2, 1))              # [32, C*8]
                idx16[core, s, 32 * q:32 * q + 32, :C * 8] = band

    meta = {
        "caps": caps, "st": st, "n16max": n16max, "rows_all": rows_all,
    }
    return idx16, npad, numf_r, meta


def _build(caps, st, n16max):
    import concourse.bass as bass
    import concourse.mybir as mybir
    import concourse.tile as tile
    from concourse import bacc
    from contextlib import ExitStack

    f32 = mybir.dt.float32
    bf16 = mybir.dt.bfloat16
    i16 = mybir.dt.int16
    AL = mybir.AluOpType
    AF = mybir.ActivationFunctionType

    STMAX = int(max(st))
    NCHMAX = STMAX // 2

    nc = bacc.Bacc(None, num_swdge_queues=4, dynamic_dma_scratch_size=32768)

    idx_d = nc.declare_dram_parameter("idx16", [NBLK, 128, n16max], i16,
                                      isOutput=False)
    npad_d = nc.declare_dram_parameter("npad", [NBLK, D, BLK], f32,
                                       isOutput=False)
    numf_d = nc.declare_dram_parameter("numfr", [NBLK, 1, BLK], f32,
                                       isOutput=False)
    table_d = nc.declare_dram_parameter("table2", [NBUCK * TROW, D], f32,
                                        isOutput=False)
    ident_d = nc.declare_dram_parameter("ident128", [128, 128], f32,
                                        isOutput=False)
    segf_d = nc.declare_dram_parameter("seg_f", [128, D], f32, isOutput=False)
    segb_d = nc.declare_dram_parameter("seg_b", [128, D], bf16, isOutput=False)
    i64b_d = nc.declare_dram_parameter("i64_b", [D, D], bf16, isOutput=False)
    i64f_d = nc.declare_dram_parameter("i64_f", [D, D], f32, isOutput=False)
    waug_d = nc.declare_dram_parameter("waug", [D + 1, 128], f32,
                                       isOutput=False)
    gw2_d = nc.declare_dram_parameter("gw2", [128, 128], f32, isOutput=False)
    g0_d = nc.declare_dram_parameter("g0T", [D, D], f32, isOutput=False)
    g1_d = nc.declare_dram_parameter("g1aug", [D + 1, D], f32, isOutput=False)
    l0_d = nc.declare_dram_parameter("l0T", [D, D], f32, isOutput=False)
    l1_d = nc.declare_dram_parameter("l1aug", [D + 1, D], f32, isOutput=False)
    cols_d = nc.declare_dram_parameter("cols", [D, 4], f32, isOutput=False)
    ones_d = nc.declare_dram_parameter("ones164", [1, D], f32, isOutput=False)
    out_d = nc.declare_dram_parameter("out", [BS, D], f32, isOutput=True)

    with tile.TileContext(nc) as tc, ExitStack() as ctx:
        const = ctx.enter_context(tc.tile_pool(name="const", bufs=1))
        sb = ctx.enter_context(tc.tile_pool(name="sb", bufs=2))
        pst = ctx.enter_context(tc.tile_pool(name="pst", bufs=2, space="PSUM"))
        psu = ctx.enter_context(tc.tile_pool(name="psu", bufs=2, space="PSUM"))
        pseg = ctx.enter_context(tc.tile_pool(name="pseg", bufs=1,
                                              space="PSUM"))
        pracc = ctx.enter_context(tc.tile_pool(name="pracc", bufs=1,
                                               space="PSUM"))
        psm = ctx.enter_context(tc.tile_pool(name="psm", bufs=1, space="PSUM"))

        ident_t = const.tile([128, 128], f32)
        nc.sync.dma_start(ident_t[:], ident_d[:])
        segf_t = const.tile([128, D], f32)
        nc.sync.dma_start(segf_t[:], segf_d[:])
        segb_t = const.tile([128, D], bf16)
        nc.sync.dma_start(segb_t[:], segb_d[:])
        i64b_t = const.tile([D, D], bf16)
        nc.sync.dma_start(i64b_t[:], i64b_d[:])
        i64f_t = const.tile([D, D], f32)
        nc.sync.dma_start(i64f_t[:], i64f_d[:])
        waug_t = const.tile([D + 1, 128], f32)
        nc.sync.dma_start(waug_t[:], waug_d[:])
        gw2_t = const.tile([128, 128], f32)
        nc.sync.dma_start(gw2_t[:], gw2_d[:])
        g0_t = const.tile([D, D], f32)
        nc.sync.dma_start(g0_t[:], g0_d[:])
        g1_t = const.tile([D + 1, D], f32)
        nc.sync.dma_start(g1_t[:], g1_d[:])
        l0_t = const.tile([D, D], f32)
        nc.sync.dma_start(l0_t[:], l0_d[:])
        l1_t = const.tile([D + 1, D], f32)
        nc.sync.dma_start(l1_t[:], l1_d[:])
        cols_t = const.tile([D, 4], f32)
        nc.sync.dma_start(cols_t[:], cols_d[:])
        ones_t = const.tile([1, D], f32)
        nc.sync.dma_start(ones_t[:], ones_d[:])
        onesrow_t = const.tile([1, 128], f32)
        nc.vector.memset(onesrow_t[:], 1.0)

        numw_c = cols_t[:, 0:1]
        numb_c = cols_t[:, 1:2]
        gb0_c = cols_t[:, 2:3]
        lb0_c = cols_t[:, 3:4]

        for blk in range(NBLK):
            ST = int(st[blk])
            NCHUNK = ST // 2
            GROUPS = []
            g0 = 0
            while g0 < NCHUNK:
                gn = min(4, NCHUNK - g0)
                GROUPS.append((g0, gn))
                g0 += gn

            idx_t = sb.tile([128, n16max], i16, tag="idx")
            nc.sync.dma_start(idx_t[:], idx_d[blk])
            numf_t = sb.tile([1, BLK], f32, tag="numf")
            nc.sync.dma_start(numf_t[:], numf_d[blk])
            npad64 = sb.tile([D, BLK], f32, tag="npad64")
            nc.sync.dma_start(npad64[:, :], npad_d[blk])

            # ---- 1. bucketed gathers on queues 0-3 ----
            emb = sb.tile([128, STMAX * D], f32, tag="emb")
            off = 0
            for q in range(NBUCK):
                C = int(caps[blk, q])
                if C == 0:
                    continue
                nc.gpsimd.dma_gather(
                    out_ap=emb[:, off * D:(off + C) * D].rearrange(
                        "p (s d) -> p s d", d=D
                    ),
                    in_ap=table_d[q * TROW:(q + 1) * TROW, :],
                    idxs_ap=idx_t[:, 0:C * 8],
                    num_idxs=C * BLK,
                    num_idxs_reg=C * BLK,
                    elem_size=D,
                    single_packet=False,
                    queue_num=q,
                )
                off += C

            # ---- numeric-field embedding (transposed): [64, 128] ----
            nrep = psm.tile([D, 128], f32, tag="small")
            nc.tensor.matmul(out=nrep[:], lhsT=ones_t[:], rhs=numf_t[:],
                             start=True, stop=True)
            numembT = sb.tile([D, 128], f32, tag="numembT")
            nc.scalar.activation(out=numembT[:], in_=nrep[:], func=AF.Identity,
                                 bias=numb_c, scale=numw_c)

            # ---- 2. transposes + evict; 3. squares ----
            et = sb.tile([128, NCHMAX * 256], f32, tag="et")
            etv = et[:].rearrange("p (j c) -> p j c", c=256)
            for gi, (gg0, gn) in enumerate(GROUPS):
                trp = pst.tile([128, 512], f32, tag="tr")
                for jj in range(gn):
                    j = gg0 + jj
                    nc.tensor.matmul(
                        out=trp[:, jj * 128:(jj + 1) * 128],
                        lhsT=emb[:, j * 128:(j + 1) * 128],
                        rhs=ident_t[:],
                        is_transpose=True, start=True, stop=True,
                    )
                src = trp[:, :gn * 128].rearrange("p (j c) -> p j c", c=128)
                nc.vector.tensor_copy(out=etv[:, gg0:gg0 + gn, 0:128], in_=src)
                nc.scalar.activation(
                    out=etv[:, gg0:gg0 + gn, 128:256], in_=src,
                    func=AF.Square,
                )

            # ---- 4. seg-matmuls: [sumT | sumsqT] -> [64, 256] PSUM ----
            seg = pseg.tile([D, 256], f32, tag="seg")
            for j in range(NCHUNK):
                nc.tensor.matmul(
                    out=seg[:], lhsT=segf_t[:],
                    rhs=et[:, j * 256:(j + 1) * 256],
                    start=(j == 0), stop=(j == NCHUNK - 1),
                    skip_group_check=True,
                )

            # ---- summedT (+aug ones row) ----
            saug = sb.tile([D + 1, 128], f32, tag="saug")
            nc.vector.tensor_tensor(out=saug[0:D, :], in0=seg[:, 0:128],
                                    in1=numembT[:], op=AL.add)
            nc.vector.tensor_copy(out=saug[D:D + 1, :], in_=onesrow_t[:])

            # ---- 5. u-matmuls ----
            r_buf = sb.tile([128, NCHMAX * 128], bf16, tag="rbuf")
            for gi, (gg0, gn) in enumerate(GROUPS):
                up = psu.tile([128, 512], f32, tag="u")
                saug_rep = (
                    saug[:].rearrange("p (o n) -> p o n", o=1)
                    .to_broadcast([D + 1, gn, 128])
                )
                nc.tensor.matmul(
                    out=up[:, 0:gn * 128], lhsT=waug_t[:], rhs=saug_rep,
                    start=True, stop=False, skip_group_check=True,
                )
                pp = 0
                while pp < gn:
                    pn = min(2, gn - pp)
                    rhs = et[:].rearrange("p (j c) -> p j c", c=256)[
                        :, gg0 + pp:gg0 + pp + pn, 0:128
                    ]
                    nc.tensor.matmul(
                        out=up[:, pp * 128:(pp + pn) * 128],
                        lhsT=gw2_t[:], rhs=rhs,
                        start=False, stop=True, skip_group_check=True,
                    )
                    pp += pn
                nc.scalar.activation(
                    out=r_buf[:, gg0 * 128:(gg0 + gn) * 128],
                    in_=up[:, :gn * 128], func=AF.Relu,
                )

            # num field u + relu; also relu(base) for pad correction
            unum = psm.tile([D, 128], f32, tag="small")
            nc.tensor.matmul(out=unum[:], lhsT=waug_t[:, 0:D], rhs=saug[:],
                             start=True, stop=False, skip_group_check=True)
            nc.tensor.matmul(out=unum[:], lhsT=gw2_t[0:D, 0:D], rhs=numembT[:],
                             start=False, stop=True, skip_group_check=True)
            rnum = sb.tile([D, 128], bf16, tag="rnum")
            nc.scalar.activation(out=rnum[:], in_=unum[:], func=AF.Relu)

            ubase = psm.tile([D, 128], f32, tag="small")
            nc.tensor.matmul(out=ubase[:], lhsT=waug_t[:, 0:D], rhs=saug[:],
                             start=True, stop=True)
            rbase = sb.tile([D, 128], f32, tag="rbase")
            nc.scalar.activation(out=rbase[:], in_=ubase[:], func=AF.Relu)

            # ---- racc: g_preT = sum_f relu(u_f) via DVE strided add-tree ----
            rtree = sb.tile([128, NCHMAX * 64], bf16, tag="rtree")
            H = NCHUNK // 2
            nc.vector.tensor_tensor(
                out=rtree[:, 0:H * 128], in0=r_buf[:, 0:H * 128],
                in1=r_buf[:, H * 128:2 * H * 128], op=AL.add,
            )
            if NCHUNK % 2:
                nc.vector.tensor_tensor(
                    out=rtree[:, 0:128], in0=rtree[:, 0:128],
                    in1=r_buf[:, (NCHUNK - 1) * 128:NCHUNK * 128], op=AL.add,
                )
            while H > 1:
                H2 = H // 2
                nc.vector.tensor_tensor(
                    out=rtree[:, 0:H2 * 128], in0=rtree[:, 0:H2 * 128],
                    in1=rtree[:, H2 * 128:2 * H2 * 128], op=AL.add,
                )
                if H % 2:
                    nc.vector.tensor_tensor(
                        out=rtree[:, 0:128], in0=rtree[:, 0:128],
                        in1=rtree[:, (H - 1) * 128:H * 128], op=AL.add,
                    )
                H = H2
            fgp = psm.tile([D, 128], f32, tag="small")
            nc.tensor.matmul(out=fgp[:], lhsT=segb_t[:], rhs=rtree[:, 0:128],
                             start=True, stop=True)
            gpreT = sb.tile([D, 128], f32, tag="gpreT")
            nc.vector.tensor_tensor(out=gpreT[:], in0=fgp[:], in1=rnum[:],
                                    op=AL.add)
            # pad correction: gpreT -= npad64 * relu(base)
            corr = sb.tile([D, 128], f32, tag="corr")
            nc.vector.tensor_tensor(out=corr[:], in0=rbase[:], in1=npad64[:, :],
                                    op=AL.mult)
            nc.vector.tensor_tensor(out=gpreT[:], in0=gpreT[:], in1=corr[:],
                                    op=AL.subtract)

            # ---- local branch: lT = summedT^2 - sumsqT ----
            lT = sb.tile([D, 128], f32, tag="lT")
            nc.vector.tensor_tensor(out=lT[:], in0=saug[0:D, :],
                                    in1=saug[0:D, :], op=AL.mult)
            nc.vector.tensor_tensor(out=lT[:], in0=lT[:], in1=seg[:, 128:256],
                                    op=AL.subtract)

            # ---- MLPs (transposed) ----
            h1p = psm.tile([D, 128], f32, tag="small")
            nc.tensor.matmul(out=h1p[:], lhsT=g0_t[:], rhs=gpreT[:],
                             start=True, stop=True)
            h1aug = sb.tile([D + 1, 128], f32, tag="h1aug")
            nc.scalar.activation(out=h1aug[0:D, :], in_=h1p[:], func=AF.Relu,
                                 bias=gb0_c)
            nc.vector.tensor_copy(out=h1aug[D:D + 1, :], in_=onesrow_t[:])

            l1p = psm.tile([D, 128], f32, tag="small")
            nc.tensor.matmul(out=l1p[:], lhsT=l0_t[:], rhs=lT[:],
                             start=True, stop=True)
            l1aug = sb.tile([D + 1, 128], f32, tag="l1aug")
            nc.scalar.activation(out=l1aug[0:D, :], in_=l1p[:], func=AF.Relu,
                                 bias=lb0_c)
            nc.vector.tensor_copy(out=l1aug[D:D + 1, :], in_=onesrow_t[:])

            outp = psm.tile([D, 128], f32, tag="small")
            nc.tensor.matmul(out=outp[:], lhsT=g1_t[:], rhs=h1aug[:],
                             start=True, stop=False, skip_group_check=True)
            nc.tensor.matmul(out=outp[:], lhsT=l1_t[:], rhs=l1aug[:],
                             start=False, stop=True, skip_group_check=True)
            outT = sb.tile([D, 128], f32, tag="outT")
            nc.scalar.copy(out=outT[:], in_=outp[:])

            finp = psm.tile([128, D], f32, tag="smallo")
            nc.tensor.matmul(out=finp[:], lhsT=outT[:], rhs=i64f_t[:],
                             is_transpose=True, start=True, stop=True)
            orow = sb.tile([128, D], f32, tag="orow")
            nc.vector.tensor_copy(out=orow[:], in_=finp[:])
            nc.sync.dma_start(out_d[blk * BLK:(blk + 1) * BLK, :], orow[:])

    return nc


def _make_consts(embed_table, num_W, num_b, ga_W, ga_b, gW, gb, lW, lb):
    """Host-side constant prep. Returns dict of name -> np.ndarray."""
    f = np.float32
    ga_W = ga_W.astype(f)
    table = np.asarray(embed_table).astype(f)
    table2 = np.zeros((NBUCK * TROW, D), f)
    for q in range(NBUCK):
        table2[q * TROW] = 0.0
        table2[q * TROW + 1:(q + 1) * TROW] = table[q * BUCK:(q + 1) * BUCK]
    ident128 = np.eye(128, dtype=f)
    i64 = np.eye(D, dtype=f)
    seg = np.vstack([i64, i64]).astype(f)           # [128, 64]
    waug = np.zeros((D + 1, 128), f)                # bias matmul lhsT
    waug[:D, :D] = ga_W / CD
    waug[:D, D:] = ga_W / CD
    waug[D, :D] = ga_b
    waug[D, D:] = ga_b
    gw2 = np.zeros((128, 128), f)                   # blockdiag support lhsT
    gw2[:D, :D] = ga_W * (PROBE / CD)
    gw2[D:, D:] = ga_W * (PROBE / CD)
    g0T = (gW[0].astype(f) / NF).T.copy()           # fold 1/51 mean
    g1aug = np.zeros((D + 1, D), f)
    g1aug[:D] = ALPHA * gW[1].astype(f).T
    g1aug[D] = ALPHA * gb[1].astype(f)
    l0T = (0.5 * lW[0].astype(f)).T.copy()          # fold FM 0.5
    l1aug = np.zeros((D + 1, D), f)
    l1aug[:D] = (1.0 - ALPHA) * lW[1].astype(f).T
    l1aug[D] = (1.0 - ALPHA) * lb[1].astype(f)
    cols = np.stack(
        [num_W[:, 0].astype(f), num_b.astype(f), gb[0].astype(f),
         lb[0].astype(f)], axis=1,
    ).copy()
    return {
        "table2": table2,
        "ident128": ident128,
        "seg_f": seg,
        "seg_b": seg,
        "i64_b": i64,
        "i64_f": i64,
        "waug": waug,
        "gw2": gw2,
        "g0T": g0T,
        "g1aug": g1aug,
        "l0T": l0T,
        "l1aug": l1aug,
        "cols": cols,
        "ones164": np.ones((1, D), f),
    }


def prepare(inputs):
    """Build (cached) nc + per-core in_maps + meta from FULL inputs."""
    import ml_dtypes

    idx16, npad, numf_r, meta = _host_prep(
        inputs["cat_indices"], inputs["num_features"]
    )
    consts = _make_consts(
        inputs["embed_table"], inputs["num_W"], inputs["num_b"],
        inputs["ga_W"], inputs["ga_b"], inputs["gW"], inputs["gb"],
        inputs["lW"], inputs["lb"],
    )
    bf = ml_dtypes.bfloat16
    cmap = {
        k: (v.astype(bf) if k in ("seg_b", "i64_b") else v)
        for k, v in consts.items()
    }

    key = (tuple(meta["caps"].ravel().tolist()), meta["n16max"])
    if _CACHE.get("key") != key:
        print("[kernel] building bass module...", flush=True)
        nc = _build(meta["caps"], meta["st"], meta["n16max"])
        print("[kernel] finalizing...", flush=True)
        nc.finalize()
        _CACHE.update({"nc": nc, "key": key})
        print("[kernel] build done", flush=True)
    nc = _CACHE["nc"]

    in_maps = []
    for c in range(NCORES):
        m = dict(cmap)
        m["idx16"] = np.ascontiguousarray(idx16[c])
        m["npad"] = np.ascontiguousarray(npad[c])
        m["numfr"] = np.ascontiguousarray(numf_r[c])
        in_maps.append(m)
    return nc, in_maps, meta


def kernel(cat_indices, num_features, embed_table, num_W, num_b,
           ga_W, ga_b, gW, gb, lW, lb):
    from concourse.bass_utils import run_bass_kernel_spmd

    nc, in_maps, meta = prepare(dict(
        cat_indices=cat_indices, num_features=num_features,
        embed_table=embed_table, num_W=num_W, num_b=num_b,
        ga_W=ga_W, ga_b=ga_b, gW=gW, gb=gb, lW=lW, lb=lb,
    ))

    print("[kernel] launching spmd run...", flush=True)
    res = run_bass_kernel_spmd(nc, in_maps, list(range(NCORES)))
    print("[kernel] run complete", flush=True)
    out = np.empty((B, D), np.float32)
    for c in range(NCORES):
        out[meta["rows_all"][c]] = res.results[c]["out"]
    return out


# revision 12
# speedup vs baseline: 1.4167x; 1.0123x over previous
"""Trainium2 Bass kernel for nn_CatEmbedder (gnn_message_passing).

Takes FULL inputs, shards batch B=32768 across 8 NeuronCores (4096 each),
replicates the embedding table + weights, runs an SPMD Bass kernel, and
concatenates + un-permutes the per-core outputs.

Gather strategy (the bottleneck): instead of 50 per-field indirect DMAs
per 128-sample block (~1us Pool-engine SWDGE time each), use 4 dma_gather
instructions per block on SWDGE queues 0-3 (concurrent Q7 cpu pairs).
dma_gather takes int16 indices, so the 100k-row table is split into 4
range-buckets of 25000 rows; each sample's 50 indices are pre-sorted so
its bucket-q rows occupy a contiguous run of its slots. Per-sample counts
vary, so each bucket region is padded to a per-slot capacity with
pointers to a zero row (prepended per bucket in a rebuilt table); zero
rows are sum/sumsq-neutral and their relu(base) contribution is
subtracted exactly via a per-sample pad-count correction.

Samples are globally sorted by bucket-count profile and dealt round-robin
to (core, slot) so all 8 cores share one NEFF with tight per-slot
capacities.

Per-core pipeline per block (ST = padded stripes, ~56-62):
  1. 4x dma_gather (queues 0-3, single_packet=False) -> emb [128, ST*64]
  2. PE transposes of [128,128] chunks -> PSUM -> SBUF (et)
  3. squares (ACT/DVE split) into interleaved sq chunks
  4. PE seg-matmuls: field-sum + field-sum-of-squares -> [64,256] PSUM
  5. PE u-matmuls: u_f = (S + PROBE*support_f)/c + ga_b (bias via aug row)
  6. ACT relu-evict (bf16) -> PE accumulates sum_f relu(u_f); subtract
     npad * relu(base) pad correction
  7. transposed MLPs for global/local branches, combine, transpose, store
"""

import os
import sys
import numpy as np

sys.path.insert(0, "/opt/trn_rl_repo")

# ---- problem constants (hardcoded per the contract) ----
B, F, D, NCT = 32768, 50, 64, 100000
PROBE, ALPHA = 39.0, 0.5
NF = F + 1              # 51 fields
CD = NF + PROBE         # 90.0
NCORES = 8
BS = B // NCORES        # 4096 samples per core
BLK = 128
NBLK = BS // BLK        # 32 blocks (slots) per core
NBUCK = 4
BUCK = NCT // NBUCK     # 25000 rows per bucket
TROW = BUCK + 1         # bucket stride in table2 (incl. zero row)

_CACHE = {}


def _host_prep(cat_indices, num_features):
    """Sort/cluster/bucket the indices. Returns per-core tensors + meta."""
    idx = np.asarray(cat_indices).astype(np.int64)
    numf = np.asarray(num_features).astype(np.float32)

    idx_sorted = np.sort(idx, axis=1)                        # [B, 50]
    # bucket counts per sample
    c = np.stack([
        (idx_sorted < BUCK).sum(1),
        ((idx_sorted >= BUCK) & (idx_sorted < 2 * BUCK)).sum(1),
        ((idx_sorted >= 2 * BUCK) & (idx_sorted < 3 * BUCK)).sum(1),
        (idx_sorted >= 3 * BUCK).sum(1),
    ], axis=1)                                               # [B, 4]
    o = np.concatenate([np.zeros((B, 1), np.int64),
                        np.cumsum(c, axis=1)], axis=1)       # [B, 5]

    # cluster sample profiles via recursive bisection so each slot's 1024
    # samples (8 cores x 128) have tight per-bucket count maxima
    def bisect(ordr, keys, splits):
        if not splits:
            return [ordr]
        o = ordr[np.argsort(c[ordr, keys[0]], kind="stable")]
        out = []
        for ch in np.array_split(o, splits[0]):
            out += bisect(ch, keys[1:], splits[1:])
        return out

    order = np.concatenate(
        bisect(np.arange(B), (3, 0, 1, 2), (2, 4, 2, 2))
    )
    # blocks of 128 consecutive samples; slot s gets blocks s*8+core
    blocks = order.reshape(B // BLK, BLK)                    # [256, 128]
    cb = np.stack([c[blocks, q].max(axis=1) for q in range(NBUCK)], axis=1)

    caps = np.zeros((NBLK, NBUCK), np.int64)
    for s in range(NBLK):
        caps[s] = cb[s * NCORES:(s + 1) * NCORES].max(axis=0)
        if caps[s].sum() % 2:
            caps[s, 0] += 1                                  # even stripes
    st = caps.sum(axis=1)                                    # [NBLK]
    n16 = caps * BLK // 16                                   # [NBLK, NBUCK]
    n16max = int(n16.max())

    idx16 = np.zeros((NCORES, NBLK, 128, n16max), np.int16)
    npad = np.zeros((NCORES, NBLK, 64, BLK), np.float32)
    numf_r = np.zeros((NCORES, NBLK, 1, BLK), np.float32)
    rows_all = np.zeros((NCORES, BS), np.int64)

    for s in range(NBLK):
        for core in range(NCORES):
            samp = blocks[s * NCORES + core]                 # [128] sample ids
            rows_all[core, s * BLK:(s + 1) * BLK] = samp
            numf_r[core, s, 0] = numf[samp]
            npad[core, s, :, :] = (caps[s][None, :] - c[samp]).sum(axis=1)[None, :]
            for q in range(NBUCK):
                C = int(caps[s, q])
                if C == 0:
                    continue
                # vals[p, k] = sorted idx (local+1) or 0 pad
                k = np.arange(C)[None, :]                    # [1, C]
                cpq = c[samp, q][:, None]                    # [128, 1]
                opq = o[samp, q][:, None]
                take = np.clip(opq + k, 0, F - 1)
                v = idx_sorted[samp[:, None], take] - q * BUCK + 1
                vals = np.where(k < cpq, v, 0).astype(np.int16)  # [128, C]
                flat = vals.T.ravel()                        # j = k*128+p
                wrapped = flat.reshape(-1, 16).T             # [16, C*8]
                band = np.tile(wrapped, (2, 1))              # [32, C*8]
                idx16[core, s, 32 * q:32 * q + 32, :C * 8] = band

    meta = {
        "caps": caps, "st": st, "n16max": n16max, "rows_all": rows_all,
    }
    return idx16, npad, numf_r, meta


def _build(caps, st, n16max):
    import concourse.bass as bass
    import concourse.mybir as mybir
    import concourse.tile as tile
    from concourse import bacc
    from contextlib import ExitStack

    f32 = mybir.dt.float32
    bf16 = mybir.dt.bfloat16
    i16 = mybir.dt.int16
    AL = mybir.AluOpType
    AF = mybir.ActivationFunctionType

    STMAX = int(max(st))
    NCHMAX = STMAX // 2

    nc = bacc.Bacc(None, num_swdge_queues=4, dynamic_dma_scratch_size=32768)

    idx_d = nc.declare_dram_parameter("idx16", [NBLK, 128, n16max], i16,
                                      isOutput=False)
    npad_d = nc.declare_dram_parameter("npad", [NBLK, D, BLK], f32,
                                       isOutput=False)
    numf_d = nc.declare_dram_parameter("numfr", [NBLK, 1, BLK], f32,
                                       isOutput=False)
    table_d = nc.declare_dram_parameter("table2", [NBUCK * TROW, D], f32,
                                        isOutput=False)
    ident_d = nc.declare_dram_parameter("ident128", [128, 128], f32,
                                        isOutput=False)
    segf_d = nc.declare_dram_parameter("seg_f", [128, D], f32, isOutput=False)
    segb_d = nc.declare_dram_parameter("seg_b", [128, D], bf16, isOutput=False)
    i64b_d = nc.declare_dram_parameter("i64_b", [D, D], bf16, isOutput=False)
    i64f_d = nc.declare_dram_parameter("i64_f", [D, D], f32, isOutput=False)
    waug_d = nc.declare_dram_parameter("waug", [D + 1, 128], f32,
                                       isOutput=False)
    gw2_d = nc.declare_dram_parameter("gw2", [128, 128], f32, isOutput=False)
    g0_d = nc.declare_dram_parameter("g0T", [D, D], f32, isOutput=False)
    g1_d = nc.declare_dram_parameter("g1aug", [D + 1, D], f32, isOutput=False)
    l0_d = nc.declare_dram_parameter("l0T", [D, D], f32, isOutput=False)
    l1_d = nc.declare_dram_parameter("l1aug", [D + 1, D], f32, isOutput=False)
    cols_d = nc.declare_dram_parameter("cols", [D, 4], f32, isOutput=False)
    ones_d = nc.declare_dram_parameter("ones164", [1, D], f32, isOutput=False)
    out_d = nc.declare_dram_parameter("out", [BS, D], f32, isOutput=True)

    with tile.TileContext(nc) as tc, ExitStack() as ctx:
        const = ctx.enter_context(tc.tile_pool(name="const", bufs=1))
        sb = ctx.enter_context(tc.tile_pool(name="sb", bufs=2))
        pst = ctx.enter_context(tc.tile_pool(name="pst", bufs=2, space="PSUM"))
        psu = ctx.enter_context(tc.tile_pool(name="psu", bufs=3, space="PSUM"))
        pseg = ctx.enter_context(tc.tile_pool(name="pseg", bufs=1,
                                              space="PSUM"))
        psm = ctx.enter_context(tc.tile_pool(name="psm", bufs=1, space="PSUM"))

        ident_t = const.tile([128, 128], f32)
        nc.sync.dma_start(ident_t[:], ident_d[:])
        segf_t = const.tile([128, D], f32)
        nc.sync.dma_start(segf_t[:], segf_d[:])
        segb_t = const.tile([128, D], bf16)
        nc.sync.dma_start(segb_t[:], segb_d[:])
        i64b_t = const.tile([D, D], bf16)
        nc.sync.dma_start(i64b_t[:], i64b_d[:])
        i64f_t = const.tile([D, D], f32)
        nc.sync.dma_start(i64f_t[:], i64f_d[:])
        waug_t = const.tile([D + 1, 128], f32)
        nc.sync.dma_start(waug_t[:], waug_d[:])
        gw2_t = const.tile([128, 128], f32)
        nc.sync.dma_start(gw2_t[:], gw2_d[:])
        g0_t = const.tile([D, D], f32)
        nc.sync.dma_start(g0_t[:], g0_d[:])
        g1_t = const.tile([D + 1, D], f32)
        nc.sync.dma_start(g1_t[:], g1_d[:])
        l0_t = const.tile([D, D], f32)
        nc.sync.dma_start(l0_t[:], l0_d[:])
        l1_t = const.tile([D + 1, D], f32)
        nc.sync.dma_start(l1_t[:], l1_d[:])
        cols_t = const.tile([D, 4], f32)
        nc.sync.dma_start(cols_t[:], cols_d[:])
        ones_t = const.tile([1, D], f32)
        nc.sync.dma_start(ones_t[:], ones_d[:])
        onesrow_t = const.tile([1, 128], f32)
        nc.vector.memset(onesrow_t[:], 1.0)

        numw_c = cols_t[:, 0:1]
        numb_c = cols_t[:, 1:2]
        gb0_c = cols_t[:, 2:3]
        lb0_c = cols_t[:, 3:4]

        for blk in range(NBLK):
            ST = int(st[blk])
            NCHUNK = ST // 2
            GROUPS = []
            g0 = 0
            while g0 < NCHUNK:
                gn = min(4, NCHUNK - g0)
                GROUPS.append((g0, gn))
                g0 += gn

            idx_t = sb.tile([128, n16max], i16, tag="idx")
            nc.sync.dma_start(idx_t[:], idx_d[blk])
            numf_t = sb.tile([1, BLK], f32, tag="numf")
            nc.sync.dma_start(numf_t[:], numf_d[blk])
            npad64 = sb.tile([D, BLK], f32, tag="npad64")
            nc.sync.dma_start(npad64[:, :], npad_d[blk])

            # ---- 1. bucketed gathers on queues 0-3 ----
            emb = sb.tile([128, STMAX * D], f32, tag="emb")
            off = 0
            for q in range(NBUCK):
                C = int(caps[blk, q])
                if C == 0:
                    continue
                nc.gpsimd.dma_gather(
                    out_ap=emb[:, off * D:(off + C) * D].rearrange(
                        "p (s d) -> p s d", d=D
                    ),
                    in_ap=table_d[q * TROW:(q + 1) * TROW, :],
                    idxs_ap=idx_t[:, 0:C * 8],
                    num_idxs=C * BLK,
                    num_idxs_reg=C * BLK,
                    elem_size=D,
                    single_packet=False,
                    queue_num=q,
                )
                off += C

            # ---- numeric-field embedding (transposed): [64, 128] ----
            nrep = psm.tile([D, 128], f32, tag="small")
            nc.tensor.matmul(out=nrep[:], lhsT=ones_t[:], rhs=numf_t[:],
                             start=True, stop=True)
            numembT = sb.tile([D, 128], f32, tag="numembT")
            nc.scalar.activation(out=numembT[:], in_=nrep[:], func=AF.Identity,
                                 bias=numb_c, scale=numw_c)

            # ---- 2. transposes + evict; 3. squares ----
            et = sb.tile([128, NCHMAX * 256], f32, tag="et")
            etv = et[:].rearrange("p (j c) -> p j c", c=256)
            for gi, (gg0, gn) in enumerate(GROUPS):
                trp = pst.tile([128, 512], f32, tag="tr")
                for jj in range(gn):
                    j = gg0 + jj
                    nc.tensor.matmul(
                        out=trp[:, jj * 128:(jj + 1) * 128],
                        lhsT=emb[:, j * 128:(j + 1) * 128],
                        rhs=ident_t[:],
                        is_transpose=True, start=True, stop=True,
                    )
                src = trp[:, :gn * 128].rearrange("p (j c) -> p j c", c=128)
                nc.vector.tensor_copy(out=etv[:, gg0:gg0 + gn, 0:128], in_=src)
                nc.scalar.activation(
                    out=etv[:, gg0:gg0 + gn, 128:256], in_=src,
                    func=AF.Square,
                )

            # ---- 4. seg-matmuls: two-chunk pairs, N=512 -> [64, 512] PSUM ----
            seg = pseg.tile([D, 512], f32, tag="seg")
            NPAIR = NCHUNK // 2
            odd = NCHUNK % 2
            for j in range(NPAIR):
                nc.tensor.matmul(
                    out=seg[:], lhsT=segf_t[:],
                    rhs=et[:, j * 512:(j + 1) * 512],
                    start=(j == 0), stop=(j == NPAIR - 1 and not odd),
                    skip_group_check=True,
                )
            if odd:
                nc.tensor.matmul(
                    out=seg[:, 0:256], lhsT=segf_t[:],
                    rhs=et[:, (NCHUNK - 1) * 256:NCHUNK * 256],
                    start=(NPAIR == 0), stop=True, skip_group_check=True,
                )
            # fold the two pair-halves: sum = seg[:,0:128]+seg[:,256:384];
            # sumsq = seg[:,128:256]+seg[:,384:512]
            ssq = sb.tile([D, 256], f32, tag="ssq")
            nc.vector.tensor_copy(out=ssq[:], in_=seg[:, 0:256])
            nc.vector.tensor_tensor(out=ssq[:], in0=ssq[:],
                                    in1=seg[:, 256:512], op=AL.add)

            # ---- summedT (+aug ones row) ----
            saug = sb.tile([D + 1, 128], f32, tag="saug")
            nc.vector.tensor_tensor(out=saug[0:D, :], in0=ssq[:, 0:128],
                                    in1=numembT[:], op=AL.add)
            nc.vector.tensor_copy(out=saug[D:D + 1, :], in_=onesrow_t[:])

            # ---- 5. u-matmuls ----
            r_buf = sb.tile([128, NCHMAX * 128], bf16, tag="rbuf")
            for gi, (gg0, gn) in enumerate(GROUPS):
                up = psu.tile([128, 512], f32, tag="u")
                saug_rep = (
                    saug[:].rearrange("p (o n) -> p o n", o=1)
                    .to_broadcast([D + 1, gn, 128])
                )
                nc.tensor.matmul(
                    out=up[:, 0:gn * 128], lhsT=waug_t[:], rhs=saug_rep,
                    start=True, stop=False, skip_group_check=True,
                )
                rhs = et[:].rearrange("p (j c) -> p j c", c=256)[
                    :, gg0:gg0 + gn, 0:128
                ]
                nc.tensor.matmul(
                    out=up[:, 0:gn * 128],
                    lhsT=gw2_t[:], rhs=rhs,
                    start=False, stop=True, skip_group_check=True,
                )
                nc.scalar.activation(
                    out=r_buf[:, gg0 * 128:(gg0 + gn) * 128],
                    in_=up[:, :gn * 128], func=AF.Relu,
                )

            # num field u + relu; also relu(base) for pad correction
            unum = psm.tile([D, 128], f32, tag="small")
            nc.tensor.matmul(out=unum[:], lhsT=waug_t[:, 0:D], rhs=saug[:],
                             start=True, stop=False, skip_group_check=True)
            nc.tensor.matmul(out=unum[:], lhsT=gw2_t[0:D, 0:D], rhs=numembT[:],
                             start=False, stop=True, skip_group_check=True)
            rnum = sb.tile([D, 128], bf16, tag="rnum")
            nc.scalar.activation(out=rnum[:], in_=unum[:], func=AF.Relu)

            ubase = psm.tile([D, 128], f32, tag="small")
            nc.tensor.matmul(out=ubase[:], lhsT=waug_t[:, 0:D], rhs=saug[:],
                             start=True, stop=True)
            rbase = sb.tile([D, 128], f32, tag="rbase")
            nc.scalar.activation(out=rbase[:], in_=ubase[:], func=AF.Relu)

            # ---- racc: g_preT = sum_f relu(u_f) via DVE strided add-tree ----
            rtree = sb.tile([128, NCHMAX * 64], bf16, tag="rtree")
            H = NCHUNK // 2
            nc.vector.tensor_tensor(
                out=rtree[:, 0:H * 128], in0=r_buf[:, 0:H * 128],
                in1=r_buf[:, H * 128:2 * H * 128], op=AL.add,
            )
            if NCHUNK % 2:
                nc.vector.tensor_tensor(
                    out=rtree[:, 0:128], in0=rtree[:, 0:128],
                    in1=r_buf[:, (NCHUNK - 1) * 128:NCHUNK * 128], op=AL.add,
                )
            while H > 1:
                H2 = H // 2
                nc.vector.tensor_tensor(
                    out=rtree[:, 0:H2 * 128], in0=rtree[:, 0:H2 * 128],
                    in1=rtree[:, H2 * 128:2 * H2 * 128], op=AL.add,
                )
                if H % 2:
                    nc.vector.tensor_tensor(
                        out=rtree[:, 0:128], in0=rtree[:, 0:128],
                        in1=rtree[:, (H - 1) * 128:H * 128], op=AL.add,
                    )
                H = H2
            fgp = psm.tile([D, 128], f32, tag="small")
            nc.tensor.matmul(out=fgp[:], lhsT=segb_t[:], rhs=rtree[:, 0:128],
                             start=True, stop=True)
            gpreT = sb.tile([D, 128], f32, tag="gpreT")
            nc.vector.tensor_tensor(out=gpreT[:], in0=fgp[:], in1=rnum[:],
                                    op=AL.add)
            # pad correction: gpreT -= npad64 * relu(base)
            corr = sb.tile([D, 128], f32, tag="corr")
            nc.vector.tensor_tensor(out=corr[:], in0=rbase[:], in1=npad64[:, :],
                                    op=AL.mult)
            nc.vector.tensor_tensor(out=gpreT[:], in0=gpreT[:], in1=corr[:],
                                    op=AL.subtract)

            # ---- local branch: lT = summedT^2 - sumsqT ----
            lT = sb.tile([D, 128], f32, tag="lT")
            nc.vector.tensor_tensor(out=lT[:], in0=saug[0:D, :],
                                    in1=saug[0:D, :], op=AL.mult)
            nc.vector.tensor_tensor(out=lT[:], in0=lT[:], in1=ssq[:, 128:256],
                                    op=AL.subtract)

            # ---- MLPs (transposed) ----
            h1p = psm.tile([D, 128], f32, tag="small")
            nc.tensor.matmul(out=h1p[:], lhsT=g0_t[:], rhs=gpreT[:],
                             start=True, stop=True)
            h1aug = sb.tile([D + 1, 128], f32, tag="h1aug")
            nc.scalar.activation(out=h1aug[0:D, :], in_=h1p[:], func=AF.Relu,
                                 bias=gb0_c)
            nc.vector.tensor_copy(out=h1aug[D:D + 1, :], in_=onesrow_t[:])

            l1p = psm.tile([D, 128], f32, tag="small")
            nc.tensor.matmul(out=l1p[:], lhsT=l0_t[:], rhs=lT[:],
                             start=True, stop=True)
            l1aug = sb.tile([D + 1, 128], f32, tag="l1aug")
            nc.scalar.activation(out=l1aug[0:D, :], in_=l1p[:], func=AF.Relu,
                                 bias=lb0_c)
            nc.vector.tensor_copy(out=l1aug[D:D + 1, :], in_=onesrow_t[:])

            outp = psm.tile([D, 128], f32, tag="small")
            nc.tensor.matmul(out=outp[:], lhsT=g1_t[:], rhs=h1aug[:],
                             start=True, stop=False, skip_group_check=True)
            nc.tensor.matmul(out=outp[:], lhsT=l1_t[:], rhs=l1aug[:],
                             start=False, stop=True, skip_group_check=True)
            outT = sb.tile([D, 128], f32, tag="outT")
            nc.scalar.copy(out=outT[:], in_=outp[:])

            finp = psm.tile([128, D], f32, tag="smallo")
            nc.tensor.matmul(out=finp[:], lhsT=outT[:], rhs=i64f_t[:],
                             is_transpose=True, start=True, stop=True)
            orow = sb.tile([128, D], f32, tag="orow")
            nc.vector.tensor_copy(out=orow[:], in_=finp[:])
            nc.sync.dma_start(out_d[blk * BLK:(blk + 1) * BLK, :], orow[:])

    return nc


def _make_consts(embed_table, num_W, num_b, ga_W, ga_b, gW, gb, lW, lb):
    """Host-side constant prep. Returns dict of name -> np.ndarray."""
    f = np.float32
    ga_W = ga_W.astype(f)
    table = np.asarray(embed_table).astype(f)
    table2 = np.zeros((NBUCK * TROW, D), f)
    for q in range(NBUCK):
        table2[q * TROW] = 0.0
        table2[q * TROW + 1:(q + 1) * TROW] = table[q * BUCK:(q + 1) * BUCK]
    ident128 = np.eye(128, dtype=f)
    i64 = np.eye(D, dtype=f)
    seg = np.vstack([i64, i64]).astype(f)           # [128, 64]
    waug = np.zeros((D + 1, 128), f)                # bias matmul lhsT
    waug[:D, :D] = ga_W / CD
    waug[:D, D:] = ga_W / CD
    waug[D, :D] = ga_b
    waug[D, D:] = ga_b
    gw2 = np.zeros((128, 128), f)                   # blockdiag support lhsT
    gw2[:D, :D] = ga_W * (PROBE / CD)
    gw2[D:, D:] = ga_W * (PROBE / CD)
    g0T = (gW[0].astype(f) / NF).T.copy()           # fold 1/51 mean
    g1aug = np.zeros((D + 1, D), f)
    g1aug[:D] = ALPHA * gW[1].astype(f).T
    g1aug[D] = ALPHA * gb[1].astype(f)
    l0T = (0.5 * lW[0].astype(f)).T.copy()          # fold FM 0.5
    l1aug = np.zeros((D + 1, D), f)
    l1aug[:D] = (1.0 - ALPHA) * lW[1].astype(f).T
    l1aug[D] = (1.0 - ALPHA) * lb[1].astype(f)
    cols = np.stack(
        [num_W[:, 0].astype(f), num_b.astype(f), gb[0].astype(f),
         lb[0].astype(f)], axis=1,
    ).copy()
    return {
        "table2": table2,
        "ident128": ident128,
        "seg_f": seg,
        "seg_b": seg,
        "i64_b": i64,
        "i64_f": i64,
        "waug": waug,
        "gw2": gw2,
        "g0T": g0T,
        "g1aug": g1aug,
        "l0T": l0T,
        "l1aug": l1aug,
        "cols": cols,
        "ones164": np.ones((1, D), f),
    }


def prepare(inputs):
    """Build (cached) nc + per-core in_maps + meta from FULL inputs."""
    import ml_dtypes

    idx16, npad, numf_r, meta = _host_prep(
        inputs["cat_indices"], inputs["num_features"]
    )
    consts = _make_consts(
        inputs["embed_table"], inputs["num_W"], inputs["num_b"],
        inputs["ga_W"], inputs["ga_b"], inputs["gW"], inputs["gb"],
        inputs["lW"], inputs["lb"],
    )
    bf = ml_dtypes.bfloat16
    cmap = {
        k: (v.astype(bf) if k in ("seg_b", "i64_b") else v)
        for k, v in consts.items()
    }

    key = (tuple(meta["caps"].ravel().tolist()), meta["n16max"])
    if _CACHE.get("key") != key:
        print("[kernel] building bass module...", flush=True)
        nc = _build(meta["caps"], meta["st"], meta["n16max"])
        print("[kernel] finalizing...", flush=True)
        nc.finalize()
        _CACHE.update({"nc": nc, "key": key})
        print("[kernel] build done", flush=True)
    nc = _CACHE["nc"]

    in_maps = []
    for c in range(NCORES):
        m = dict(cmap)
        m["idx16"] = np.ascontiguousarray(idx16[c])
        m["npad"] = np.ascontiguousarray(npad[c])
        m["numfr"] = np.ascontiguousarray(numf_r[c])
        in_maps.append(m)
    return nc, in_maps, meta


def kernel(cat_indices, num_features, embed_table, num_W, num_b,
           ga_W, ga_b, gW, gb, lW, lb):
    from concourse.bass_utils import run_bass_kernel_spmd

    nc, in_maps, meta = prepare(dict(
        cat_indices=cat_indices, num_features=num_features,
        embed_table=embed_table, num_W=num_W, num_b=num_b,
        ga_W=ga_W, ga_b=ga_b, gW=gW, gb=gb, lW=lW, lb=lb,
    ))

    print("[kernel] launching spmd run...", flush=True)
    res = run_bass_kernel_spmd(nc, in_maps, list(range(NCORES)))
    print("[kernel] run complete", flush=True)
    out = np.empty((B, D), np.float32)
    for c in range(NCORES):
        out[meta["rows_all"][c]] = res.results[c]["out"]
    return out


# revision 13
# speedup vs baseline: 1.4678x; 1.0361x over previous
"""Trainium2 Bass kernel for nn_CatEmbedder (gnn_message_passing).

Takes FULL inputs, shards batch B=32768 across 8 NeuronCores (4096 each),
replicates the embedding table + weights, runs an SPMD Bass kernel, and
concatenates + un-permutes the per-core outputs.

Gather strategy (the bottleneck): instead of 50 per-field indirect DMAs
per 128-sample block (~1us Pool-engine SWDGE time each), use 4 dma_gather
instructions per block on SWDGE queues 0-3 (concurrent Q7 cpu pairs).
dma_gather takes int16 indices, so the 100k-row table is split into 4
range-buckets of 25000 rows; each sample's 50 indices are pre-sorted so
its bucket-q rows occupy a contiguous run of its slots. Per-sample counts
vary, so each bucket region is padded to a per-slot capacity with
pointers to a zero row (prepended per bucket in a rebuilt table); zero
rows are sum/sumsq-neutral and their relu(base) contribution is
subtracted exactly via a per-sample pad-count correction.

Samples are globally sorted by bucket-count profile and dealt round-robin
to (core, slot) so all 8 cores share one NEFF with tight per-slot
capacities.

Per-core pipeline per block (ST = padded stripes, ~56-62):
  1. 4x dma_gather (queues 0-3, single_packet=False) -> emb [128, ST*64]
  2. PE transposes of [128,128] chunks -> PSUM -> SBUF (et)
  3. squares (ACT/DVE split) into interleaved sq chunks
  4. PE seg-matmuls: field-sum + field-sum-of-squares -> [64,256] PSUM
  5. PE u-matmuls: u_f = (S + PROBE*support_f)/c + ga_b (bias via aug row)
  6. ACT relu-evict (bf16) -> PE accumulates sum_f relu(u_f); subtract
     npad * relu(base) pad correction
  7. transposed MLPs for global/local branches, combine, transpose, store
"""

import os
import sys
import numpy as np

sys.path.insert(0, "/opt/trn_rl_repo")

# ---- problem constants (hardcoded per the contract) ----
B, F, D, NCT = 32768, 50, 64, 100000
PROBE, ALPHA = 39.0, 0.5
NF = F + 1              # 51 fields
CD = NF + PROBE         # 90.0
NCORES = 8
BS = B // NCORES        # 4096 samples per core
BLK = 128
NBLK = BS // BLK        # 32 blocks (slots) per core
NBUCK = 4
BUCK = NCT // NBUCK     # 25000 rows per bucket
TROW = BUCK + 1         # bucket stride in table2 (incl. zero row)

_CACHE = {}


def _host_prep(cat_indices, num_features):
    """Sort/cluster/bucket the indices. Returns per-core tensors + meta."""
    idx = np.asarray(cat_indices).astype(np.int64)
    numf = np.asarray(num_features).astype(np.float32)

    idx_sorted = np.sort(idx, axis=1)                        # [B, 50]
    # bucket counts per sample
    c = np.stack([
        (idx_sorted < BUCK).sum(1),
        ((idx_sorted >= BUCK) & (idx_sorted < 2 * BUCK)).sum(1),
        ((idx_sorted >= 2 * BUCK) & (idx_sorted < 3 * BUCK)).sum(1),
        (idx_sorted >= 3 * BUCK).sum(1),
    ], axis=1)                                               # [B, 4]
    o = np.concatenate([np.zeros((B, 1), np.int64),
                        np.cumsum(c, axis=1)], axis=1)       # [B, 5]

    # cluster sample profiles via recursive bisection so each slot's 1024
    # samples (8 cores x 128) have tight per-bucket count maxima
    def bisect(ordr, keys, splits):
        if not splits:
            return [ordr]
        o = ordr[np.argsort(c[ordr, keys[0]], kind="stable")]
        out = []
        for ch in np.array_split(o, splits[0]):
            out += bisect(ch, keys[1:], splits[1:])
        return out

    order = np.concatenate(
        bisect(np.arange(B), (3, 0, 1, 2), (2, 4, 2, 2))
    )
    # blocks of 128 consecutive samples; slot s gets blocks s*8+core
    blocks = order.reshape(B // BLK, BLK)                    # [256, 128]
    cb = np.stack([c[blocks, q].max(axis=1) for q in range(NBUCK)], axis=1)

    caps = np.zeros((NBLK, NBUCK), np.int64)
    for s in range(NBLK):
        caps[s] = cb[s * NCORES:(s + 1) * NCORES].max(axis=0)
        if caps[s].sum() % 2:
            caps[s, 0] += 1                                  # even stripes
    st = caps.sum(axis=1)                                    # [NBLK]
    n16 = caps * BLK // 16                                   # [NBLK, NBUCK]
    n16max = int(n16.max())

    idx16 = np.zeros((NCORES, NBLK, 128, n16max), np.int16)
    npad = np.zeros((NCORES, NBLK, 64, BLK), np.float32)
    numf_r = np.zeros((NCORES, NBLK, 1, BLK), np.float32)
    rows_all = np.zeros((NCORES, BS), np.int64)

    for s in range(NBLK):
        for core in range(NCORES):
            samp = blocks[s * NCORES + core]                 # [128] sample ids
            rows_all[core, s * BLK:(s + 1) * BLK] = samp
            numf_r[core, s, 0] = numf[samp]
            npad[core, s, :, :] = (caps[s][None, :] - c[samp]).sum(axis=1)[None, :]
            for q in range(NBUCK):
                C = int(caps[s, q])
                if C == 0:
                    continue
                # vals[p, k] = sorted idx (local+1) or 0 pad
                k = np.arange(C)[None, :]                    # [1, C]
                cpq = c[samp, q][:, None]                    # [128, 1]
                opq = o[samp, q][:, None]
                take = np.clip(opq + k, 0, F - 1)
                v = idx_sorted[samp[:, None], take] - q * BUCK + 1
                vals = np.where(k < cpq, v, 0).astype(np.int16)  # [128, C]
                flat = vals.T.ravel()                        # j = k*128+p
                wrapped = flat.reshape(-1, 16).T             # [16, C*8]
                band = np.tile(wrapped, (2, 1))              # [32, C*8]
                idx16[core, s, 32 * q:32 * q + 32, :C * 8] = band

    meta = {
        "caps": caps, "st": st, "n16max": n16max, "rows_all": rows_all,
    }
    return idx16, npad, numf_r, meta


def _build(caps, st, n16max):
    import concourse.bass as bass
    import concourse.mybir as mybir
    import concourse.tile as tile
    from concourse import bacc
    from contextlib import ExitStack

    f32 = mybir.dt.float32
    bf16 = mybir.dt.bfloat16
    i16 = mybir.dt.int16
    AL = mybir.AluOpType
    AF = mybir.ActivationFunctionType

    STMAX = int(max(st))
    NCHMAX = STMAX // 2

    nc = bacc.Bacc(None, num_swdge_queues=4, dynamic_dma_scratch_size=32768)

    idx_d = nc.declare_dram_parameter("idx16", [NBLK, 128, n16max], i16,
                                      isOutput=False)
    npad_d = nc.declare_dram_parameter("npad", [NBLK, D, BLK], f32,
                                       isOutput=False)
    numf_d = nc.declare_dram_parameter("numfr", [NBLK, 1, BLK], f32,
                                       isOutput=False)
    table_d = nc.declare_dram_parameter("table2", [NBUCK * TROW, D], f32,
                                        isOutput=False)
    ident_d = nc.declare_dram_parameter("ident128", [128, 128], f32,
                                        isOutput=False)
    segf_d = nc.declare_dram_parameter("seg_f", [128, D], f32, isOutput=False)
    segb_d = nc.declare_dram_parameter("seg_b", [128, D], bf16, isOutput=False)
    i64b_d = nc.declare_dram_parameter("i64_b", [D, D], bf16, isOutput=False)
    i64f_d = nc.declare_dram_parameter("i64_f", [D, D], f32, isOutput=False)
    waug_d = nc.declare_dram_parameter("waug", [D + 1, 128], f32,
                                       isOutput=False)
    gw2_d = nc.declare_dram_parameter("gw2", [128, 128], f32, isOutput=False)
    g0_d = nc.declare_dram_parameter("g0T", [D, D], f32, isOutput=False)
    g1_d = nc.declare_dram_parameter("g1aug", [D + 1, D], f32, isOutput=False)
    l0_d = nc.declare_dram_parameter("l0T", [D, D], f32, isOutput=False)
    l1_d = nc.declare_dram_parameter("l1aug", [D + 1, D], f32, isOutput=False)
    cols_d = nc.declare_dram_parameter("cols", [D, 4], f32, isOutput=False)
    ones_d = nc.declare_dram_parameter("ones164", [1, D], f32, isOutput=False)
    out_d = nc.declare_dram_parameter("out", [BS, D], f32, isOutput=True)

    with tile.TileContext(nc) as tc, ExitStack() as ctx:
        const = ctx.enter_context(tc.tile_pool(name="const", bufs=1))
        sb = ctx.enter_context(tc.tile_pool(name="sb", bufs=2))
        sm = ctx.enter_context(tc.tile_pool(name="sm", bufs=4))
        pst = ctx.enter_context(tc.tile_pool(name="pst", bufs=3, space="PSUM"))
        psu = ctx.enter_context(tc.tile_pool(name="psu", bufs=2, space="PSUM"))
        pseg = ctx.enter_context(tc.tile_pool(name="pseg", bufs=1,
                                              space="PSUM"))
        psm = ctx.enter_context(tc.tile_pool(name="psm", bufs=1, space="PSUM"))

        ident_t = const.tile([128, 128], f32)
        nc.sync.dma_start(ident_t[:], ident_d[:])
        segf_t = const.tile([128, D], f32)
        nc.sync.dma_start(segf_t[:], segf_d[:])
        segb_t = const.tile([128, D], bf16)
        nc.sync.dma_start(segb_t[:], segb_d[:])
        i64b_t = const.tile([D, D], bf16)
        nc.sync.dma_start(i64b_t[:], i64b_d[:])
        i64f_t = const.tile([D, D], f32)
        nc.sync.dma_start(i64f_t[:], i64f_d[:])
        waug_t = const.tile([D + 1, 128], f32)
        nc.sync.dma_start(waug_t[:], waug_d[:])
        gw2_t = const.tile([128, 128], f32)
        nc.sync.dma_start(gw2_t[:], gw2_d[:])
        g0_t = const.tile([D, D], f32)
        nc.sync.dma_start(g0_t[:], g0_d[:])
        g1_t = const.tile([D + 1, D], f32)
        nc.sync.dma_start(g1_t[:], g1_d[:])
        l0_t = const.tile([D, D], f32)
        nc.sync.dma_start(l0_t[:], l0_d[:])
        l1_t = const.tile([D + 1, D], f32)
        nc.sync.dma_start(l1_t[:], l1_d[:])
        cols_t = const.tile([D, 4], f32)
        nc.sync.dma_start(cols_t[:], cols_d[:])
        ones_t = const.tile([1, D], f32)
        nc.sync.dma_start(ones_t[:], ones_d[:])
        onesrow_t = const.tile([1, 128], f32)
        nc.vector.memset(onesrow_t[:], 1.0)

        numw_c = cols_t[:, 0:1]
        numb_c = cols_t[:, 1:2]
        gb0_c = cols_t[:, 2:3]
        lb0_c = cols_t[:, 3:4]

        for blk in range(NBLK):
            ST = int(st[blk])
            NCHUNK = ST // 2
            GROUPS = []
            g0 = 0
            while g0 < NCHUNK:
                gn = min(4, NCHUNK - g0)
                GROUPS.append((g0, gn))
                g0 += gn

            idx_t = sb.tile([128, n16max], i16, tag="idx")
            nc.sync.dma_start(idx_t[:], idx_d[blk])
            numf_t = sm.tile([1, BLK], f32, tag="numf")
            nc.sync.dma_start(numf_t[:], numf_d[blk])
            npad64 = sm.tile([D, BLK], f32, tag="npad64")
            nc.sync.dma_start(npad64[:, :], npad_d[blk])

            # ---- 1. bucketed gathers on queues 0-3 ----
            emb = sb.tile([128, STMAX * D], f32, tag="emb")
            off = 0
            for q in range(NBUCK):
                C = int(caps[blk, q])
                if C == 0:
                    continue
                nc.gpsimd.dma_gather(
                    out_ap=emb[:, off * D:(off + C) * D].rearrange(
                        "p (s d) -> p s d", d=D
                    ),
                    in_ap=table_d[q * TROW:(q + 1) * TROW, :],
                    idxs_ap=idx_t[:, 0:C * 8],
                    num_idxs=C * BLK,
                    num_idxs_reg=C * BLK,
                    elem_size=D,
                    single_packet=False,
                    queue_num=q,
                )
                off += C

            # ---- numeric-field embedding (transposed): [64, 128] ----
            nrep = psm.tile([D, 128], f32, tag="small")
            nc.tensor.matmul(out=nrep[:], lhsT=ones_t[:], rhs=numf_t[:],
                             start=True, stop=True)
            numembT = sm.tile([D, 128], f32, tag="numembT")
            nc.scalar.activation(out=numembT[:], in_=nrep[:], func=AF.Identity,
                                 bias=numb_c, scale=numw_c)

            # ---- 2. transposes + evict; 3. squares ----
            et = sb.tile([128, NCHMAX * 256], f32, tag="et")
            etv = et[:].rearrange("p (j c) -> p j c", c=256)
            for gi, (gg0, gn) in enumerate(GROUPS):
                trp = pst.tile([128, 512], f32, tag="tr")
                for jj in range(gn):
                    j = gg0 + jj
                    nc.tensor.matmul(
                        out=trp[:, jj * 128:(jj + 1) * 128],
                        lhsT=emb[:, j * 128:(j + 1) * 128],
                        rhs=ident_t[:],
                        is_transpose=True, start=True, stop=True,
                    )
                src = trp[:, :gn * 128].rearrange("p (j c) -> p j c", c=128)
                nc.vector.tensor_copy(out=etv[:, gg0:gg0 + gn, 0:128], in_=src)
                nc.scalar.activation(
                    out=etv[:, gg0:gg0 + gn, 128:256], in_=src,
                    func=AF.Square,
                )

            # ---- 4. seg-matmuls: two-chunk pairs, N=512 -> [64, 512] PSUM ----
            seg = pseg.tile([D, 512], f32, tag="seg")
            NPAIR = NCHUNK // 2
            odd = NCHUNK % 2
            for j in range(NPAIR):
                nc.tensor.matmul(
                    out=seg[:], lhsT=segf_t[:],
                    rhs=et[:, j * 512:(j + 1) * 512],
                    start=(j == 0), stop=(j == NPAIR - 1 and not odd),
                    skip_group_check=True,
                )
            if odd:
                nc.tensor.matmul(
                    out=seg[:, 0:256], lhsT=segf_t[:],
                    rhs=et[:, (NCHUNK - 1) * 256:NCHUNK * 256],
                    start=(NPAIR == 0), stop=True, skip_group_check=True,
                )
            # fold the two pair-halves: sum = seg[:,0:128]+seg[:,256:384];
            # sumsq = seg[:,128:256]+seg[:,384:512]
            ssq = sm.tile([D, 256], f32, tag="ssq")
            nc.vector.tensor_copy(out=ssq[:], in_=seg[:, 0:256])
            nc.vector.tensor_tensor(out=ssq[:], in0=ssq[:],
                                    in1=seg[:, 256:512], op=AL.add)

            # ---- summedT (+aug ones row) ----
            saug = sm.tile([D + 1, 128], f32, tag="saug")
            nc.vector.tensor_tensor(out=saug[0:D, :], in0=ssq[:, 0:128],
                                    in1=numembT[:], op=AL.add)
            nc.vector.tensor_copy(out=saug[D:D + 1, :], in_=onesrow_t[:])

            # ---- 5. u-matmuls ----
            r_buf = sb.tile([128, NCHMAX * 128], bf16, tag="rbuf")
            for gi, (gg0, gn) in enumerate(GROUPS):
                up = psu.tile([128, 512], f32, tag="u")
                saug_rep = (
                    saug[:].rearrange("p (o n) -> p o n", o=1)
                    .to_broadcast([D + 1, gn, 128])
                )
                nc.tensor.matmul(
                    out=up[:, 0:gn * 128], lhsT=waug_t[:], rhs=saug_rep,
                    start=True, stop=False, skip_group_check=True,
                )
                rhs = et[:].rearrange("p (j c) -> p j c", c=256)[
                    :, gg0:gg0 + gn, 0:128
                ]
                nc.tensor.matmul(
                    out=up[:, 0:gn * 128],
                    lhsT=gw2_t[:], rhs=rhs,
                    start=False, stop=True, skip_group_check=True,
                )
                nc.scalar.activation(
                    out=r_buf[:, gg0 * 128:(gg0 + gn) * 128],
                    in_=up[:, :gn * 128], func=AF.Relu,
                )

            # num field u + relu; also relu(base) for pad correction
            unum = psm.tile([D, 128], f32, tag="small")
            nc.tensor.matmul(out=unum[:], lhsT=waug_t[:, 0:D], rhs=saug[:],
                             start=True, stop=False, skip_group_check=True)
            nc.tensor.matmul(out=unum[:], lhsT=gw2_t[0:D, 0:D], rhs=numembT[:],
                             start=False, stop=True, skip_group_check=True)
            rnum = sm.tile([D, 128], bf16, tag="rnum")
            nc.scalar.activation(out=rnum[:], in_=unum[:], func=AF.Relu)

            ubase = psm.tile([D, 128], f32, tag="small")
            nc.tensor.matmul(out=ubase[:], lhsT=waug_t[:, 0:D], rhs=saug[:],
                             start=True, stop=True)
            rbase = sm.tile([D, 128], f32, tag="rbase")
            nc.scalar.activation(out=rbase[:], in_=ubase[:], func=AF.Relu)

            # ---- racc: g_preT = sum_f relu(u_f) via DVE strided add-tree ----
            rtree = sb.tile([128, NCHMAX * 64], bf16, tag="rtree")
            H = NCHUNK // 2
            nc.vector.tensor_tensor(
                out=rtree[:, 0:H * 128], in0=r_buf[:, 0:H * 128],
                in1=r_buf[:, H * 128:2 * H * 128], op=AL.add,
            )
            if NCHUNK % 2:
                nc.vector.tensor_tensor(
                    out=rtree[:, 0:128], in0=rtree[:, 0:128],
                    in1=r_buf[:, (NCHUNK - 1) * 128:NCHUNK * 128], op=AL.add,
                )
            while H > 1:
                H2 = H // 2
                nc.vector.tensor_tensor(
                    out=rtree[:, 0:H2 * 128], in0=rtree[:, 0:H2 * 128],
                    in1=rtree[:, H2 * 128:2 * H2 * 128], op=AL.add,
                )
                if H % 2:
                    nc.vector.tensor_tensor(
                        out=rtree[:, 0:128], in0=rtree[:, 0:128],
                        in1=rtree[:, (H - 1) * 128:H * 128], op=AL.add,
                    )
                H = H2
            fgp = psm.tile([D, 128], f32, tag="small")
            nc.tensor.matmul(out=fgp[:], lhsT=segb_t[:], rhs=rtree[:, 0:128],
                             start=True, stop=True)
            gpreT = sm.tile([D, 128], f32, tag="gpreT")
            nc.vector.tensor_tensor(out=gpreT[:], in0=fgp[:], in1=rnum[:],
                                    op=AL.add)
            # pad correction: gpreT -= npad64 * relu(base)
            corr = sm.tile([D, 128], f32, tag="corr")
            nc.vector.tensor_tensor(out=corr[:], in0=rbase[:], in1=npad64[:, :],
                                    op=AL.mult)
            nc.vector.tensor_tensor(out=gpreT[:], in0=gpreT[:], in1=corr[:],
                                    op=AL.subtract)

            # ---- local branch: lT = summedT^2 - sumsqT ----
            lT = sm.tile([D, 128], f32, tag="lT")
            nc.vector.tensor_tensor(out=lT[:], in0=saug[0:D, :],
                                    in1=saug[0:D, :], op=AL.mult)
            nc.vector.tensor_tensor(out=lT[:], in0=lT[:], in1=ssq[:, 128:256],
                                    op=AL.subtract)

            # ---- MLPs (transposed) ----
            h1p = psm.tile([D, 128], f32, tag="small")
            nc.tensor.matmul(out=h1p[:], lhsT=g0_t[:], rhs=gpreT[:],
                             start=True, stop=True)
            h1aug = sm.tile([D + 1, 128], f32, tag="h1aug")
            nc.scalar.activation(out=h1aug[0:D, :], in_=h1p[:], func=AF.Relu,
                                 bias=gb0_c)
            nc.vector.tensor_copy(out=h1aug[D:D + 1, :], in_=onesrow_t[:])

            l1p = psm.tile([D, 128], f32, tag="small")
            nc.tensor.matmul(out=l1p[:], lhsT=l0_t[:], rhs=lT[:],
                             start=True, stop=True)
            l1aug = sm.tile([D + 1, 128], f32, tag="l1aug")
            nc.scalar.activation(out=l1aug[0:D, :], in_=l1p[:], func=AF.Relu,
                                 bias=lb0_c)
            nc.vector.tensor_copy(out=l1aug[D:D + 1, :], in_=onesrow_t[:])

            outp = psm.tile([D, 128], f32, tag="small")
            nc.tensor.matmul(out=outp[:], lhsT=g1_t[:], rhs=h1aug[:],
                             start=True, stop=False, skip_group_check=True)
            nc.tensor.matmul(out=outp[:], lhsT=l1_t[:], rhs=l1aug[:],
                             start=False, stop=True, skip_group_check=True)
            outT = sm.tile([D, 128], f32, tag="outT")
            nc.scalar.copy(out=outT[:], in_=outp[:])

            finp = psm.tile([128, D], f32, tag="smallo")
            nc.tensor.matmul(out=finp[:], lhsT=outT[:], rhs=i64f_t[:],
                             is_transpose=True, start=True, stop=True)
            orow = sm.tile([128, D], f32, tag="orow")
            nc.vector.tensor_copy(out=orow[:], in_=finp[:])
            nc.sync.dma_start(out_d[blk * BLK:(blk + 1) * BLK, :], orow[:])

    return nc


def _make_consts(embed_table, num_W, num_b, ga_W, ga_b, gW, gb, lW, lb):
    """Host-side constant prep. Returns dict of name -> np.ndarray."""
    f = np.float32
    ga_W = ga_W.astype(f)
    table = np.asarray(embed_table).astype(f)
    table2 = np.zeros((NBUCK * TROW, D), f)
    for q in range(NBUCK):
        table2[q * TROW] = 0.0
        table2[q * TROW + 1:(q + 1) * TROW] = table[q * BUCK:(q + 1) * BUCK]
    ident128 = np.eye(128, dtype=f)
    i64 = np.eye(D, dtype=f)
    seg = np.vstack([i64, i64]).astype(f)           # [128, 64]
    waug = np.zeros((D + 1, 128), f)                # bias matmul lhsT
    waug[:D, :D] = ga_W / CD
    waug[:D, D:] = ga_W / CD
    waug[D, :D] = ga_b
    waug[D, D:] = ga_b
    gw2 = np.zeros((128, 128), f)                   # blockdiag support lhsT
    gw2[:D, :D] = ga_W * (PROBE / CD)
    gw2[D:, D:] = ga_W * (PROBE / CD)
    g0T = (gW[0].astype(f) / NF).T.copy()           # fold 1/51 mean
    g1aug = np.zeros((D + 1, D), f)
    g1aug[:D] = ALPHA * gW[1].astype(f).T
    g1aug[D] = ALPHA * gb[1].astype(f)
    l0T = (0.5 * lW[0].astype(f)).T.copy()          # fold FM 0.5
    l1aug = np.zeros((D + 1, D), f)
    l1aug[:D] = (1.0 - ALPHA) * lW[1].astype(f).T
    l1aug[D] = (1.0 - ALPHA) * lb[1].astype(f)
    cols = np.stack(
        [num_W[:, 0].astype(f), num_b.astype(f), gb[0].astype(f),
         lb[0].astype(f)], axis=1,
    ).copy()
    return {
        "table2": table2,
        "ident128": ident128,
        "seg_f": seg,
        "seg_b": seg,
        "i64_b": i64,
        "i64_f": i64,
        "waug": waug,
        "gw2": gw2,
        "g0T": g0T,
        "g1aug": g1aug,
        "l0T": l0T,
        "l1aug": l1aug,
        "cols": cols,
        "ones164": np.ones((1, D), f),
    }


def prepare(inputs):
    """Build (cached) nc + per-core in_maps + meta from FULL inputs."""
    import ml_dtypes

    idx16, npad, numf_r, meta = _host_prep(
        inputs["cat_indices"], inputs["num_features"]
    )
    consts = _make_consts(
        inputs["embed_table"], inputs["num_W"], inputs["num_b"],
        inputs["ga_W"], inputs["ga_b"], inputs["gW"], inputs["gb"],
        inputs["lW"], inputs["lb"],
    )
    bf = ml_dtypes.bfloat16
    cmap = {
        k: (v.astype(bf) if k in ("seg_b", "i64_b") else v)
        for k, v in consts.items()
    }

    key = (tuple(meta["caps"].ravel().tolist()), meta["n16max"])
    if _CACHE.get("key") != key:
        print("[kernel] building bass module...", flush=True)
        nc = _build(meta["caps"], meta["st"], meta["n16max"])
        print("[kernel] finalizing...", flush=True)
        nc.finalize()
        _CACHE.update({"nc": nc, "key": key})
        print("[kernel] build done", flush=True)
    nc = _CACHE["nc"]

    in_maps = []
    for c in range(NCORES):
        m = dict(cmap)
        m["idx16"] = np.ascontiguousarray(idx16[c])
        m["npad"] = np.ascontiguousarray(npad[c])
        m["numfr"] = np.ascontiguousarray(numf_r[c])
        in_maps.append(m)
    return nc, in_maps, meta


def kernel(cat_indices, num_features, embed_table, num_W, num_b,
           ga_W, ga_b, gW, gb, lW, lb):
    from concourse.bass_utils import run_bass_kernel_spmd

    nc, in_maps, meta = prepare(dict(
        cat_indices=cat_indices, num_features=num_features,
        embed_table=embed_table, num_W=num_W, num_b=num_b,
        ga_W=ga_W, ga_b=ga_b, gW=gW, gb=gb, lW=lW, lb=lb,
    ))

    print("[kernel] launching spmd run...", flush=True)
    res = run_bass_kernel_spmd(nc, in_maps, list(range(NCORES)))
    print("[kernel] run complete", flush=True)
    out = np.empty((B, D), np.float32)
    for c in range(NCORES):
        out[meta["rows_all"][c]] = res.results[c]["out"]
    return out
